# revision 14
# baseline (speedup 1.0000x reference)
"""GAT message-passing kernel for 8 Trainium2 NeuronCores (Bass/Tile).

Strategy ("route edges by dst ownership", no collectives):
  - Host renumbers nodes (LPT bin-packing by degree) into 160 blocks of <=128
    nodes so every 128-node block has nearly equal in-edge count; each core
    owns 20 blocks, so segment-softmax and scatter-sum are fully core-local.
  - Reassociation: epaths = y1[src] + eft@W2 + y3[dst] + b with y1 = nft@W1.
    Since sum(att)=1 per (node, head), the y3[dst] part of the aggregated
    message is exactly +y3[dst], added once per node in phase 3.  Softmax is
    computed without max-subtraction; a fixed shift exp(a-7) keeps the
    unnormalized weights in fp16 range (softmax is shift-invariant).
  - No node-table phase and no device-side gather: the src node features are
    gathered per edge on the HOST (the sharding hint: "each device holds the
    gathered src/dst features") into a dense transposed tensor hsT that the
    kernel streams with full-rate 2KB DMA descriptors, and y1/qa are computed
    per edge on the PE:
      psEP[:, k, 0:136] = eftT_tile.T @ [W2 | W2A2] + hsT_tile.T @ [W1 | Wqa]
    where Wqa = W_attn1 + W1@A2. The r[dst] = y3@A2 logit term is tiny
    (8 floats per node) and is gathered per edge on the host (rdst); dummy
    padding edges get rdst = -1e4 which drives their weight u to exp(<-100)=0.
  - Per tile: one-hot P (dst-slot x edge) is built on-chip by a DVE is_equal
    against an iota row; P is the stationary of the scatter matmul that
    accumulates [agg | s] for the owning 128-node block.
  - Phase 3 (per node block): agg/s, transpose via PE, += nft@W3 (matmul
    accumulate, deg>0-masked) and += nft, relu, store transposed output.
"""

import sys
import heapq
import numpy as np

for _p in ("/opt/trn_rl_repo",):
    if _p not in sys.path:
        sys.path.append(_p)

import concourse.bacc as bacc
import concourse.bass as bass
import concourse.mybir as mybir
from concourse.tile import TileContext
from concourse import bass_utils

F = 128
H = 8
DH = 16
F2 = F + H  # 136
NCORES = 8
EXP_SHIFT = 7.0  # exp(a - shift); softmax-invariant, keeps u in fp16 range
BATCH = 3        # tiles per PSUM epilogue batch ([128, 3, 136] f32 <= 2KB bank)


def build_nc(npc, tpb, npad, has_bias, debug=False):
    nb = npc // 128                  # node blocks per core
    ntiles = nb * tpb                # edge tiles per core
    epad = ntiles * 128              # padded edge count per core
    dt = mybir.dt
    AOP = mybir.AluOpType

    nc = bacc.Bacc("TRN2", target_bir_lowering=False, debug=False,
                   num_devices=NCORES, num_swdge_queues=4)

    # ---- inputs ----
    eftT = nc.dram_tensor("eftT", (F, epad), dt.float16, kind="ExternalInput")
    hsT_in = nc.dram_tensor("hsT", (F, epad), dt.float16, kind="ExternalInput")
    nftT_c = nc.dram_tensor("nftT_c", (F, npc), dt.float16, kind="ExternalInput")
    nftT_cm = nc.dram_tensor("nftT_cm", (F, npc), dt.float16, kind="ExternalInput")
    w2cat_in = nc.dram_tensor("w2cat", (F, F2), dt.float16, kind="ExternalInput")
    w1qa_in = nc.dram_tensor("w1qa", (F, F2), dt.float16, kind="ExternalInput")
    w3_in = nc.dram_tensor("w3", (F, F), dt.float16, kind="ExternalInput")
    dstloc_in = nc.dram_tensor("dstloc", (128, ntiles), dt.float32, kind="ExternalInput")
    rdst_in = nc.dram_tensor("rdst", (128, ntiles * H), dt.float16, kind="ExternalInput")
    if has_bias:
        brow_in = nc.dram_tensor("brow", (1, F), dt.float16, kind="ExternalInput")

    outT = nc.dram_tensor("outT", (F, npc), dt.float32, kind="ExternalOutput")

    with TileContext(nc) as tc:
        with tc.tile_pool(name="const", bufs=1) as cpool, \
             tc.tile_pool(name="work", bufs=3) as pool, \
             tc.tile_pool(name="io", bufs=4) as iop, \
             tc.tile_pool(name="psEP", bufs=3, space="PSUM") as psEP, \
             tc.tile_pool(name="psB", bufs=2, space="PSUM") as psB, \
             tc.tile_pool(name="psC", bufs=2, space="PSUM") as psC:

            # ---------- constants ----------
            iota_row = cpool.tile([128, 128], dt.float32)
            nc.gpsimd.iota(iota_row, pattern=[[1, 128]], channel_multiplier=0,
                           allow_small_or_imprecise_dtypes=True)
            iota_col = cpool.tile([128, 1], dt.float32)
            nc.gpsimd.iota(iota_col, pattern=[[1, 1]], channel_multiplier=1,
                           allow_small_or_imprecise_dtypes=True)
            ident = cpool.tile([128, 128], dt.float32)
            nc.vector.tensor_scalar(out=ident, in0=iota_row[:, :],
                                    scalar1=iota_col[:, :], scalar2=None,
                                    op0=AOP.is_equal)
            nshift = cpool.tile([128, 1], dt.float32)
            nc.vector.memset(nshift, -EXP_SHIFT)

            w2cat_s = cpool.tile([F, F2], dt.float16)
            nc.sync.dma_start(out=w2cat_s, in_=w2cat_in[:, :])
            w1qa_s = cpool.tile([F, F2], dt.float16)
            nc.sync.dma_start(out=w1qa_s, in_=w1qa_in[:, :])
            w3_s = cpool.tile([F, F], dt.float16)
            nc.sync.dma_start(out=w3_s, in_=w3_in[:, :])

            if has_bias:
                brow_s = cpool.tile([1, F], dt.float16)
                nc.sync.dma_start(out=brow_s, in_=brow_in[:, :])
                ones_row = cpool.tile([1, 128], dt.float16)
                nc.vector.memset(ones_row, 1.0)

            dstloc_s = cpool.tile([128, ntiles], dt.float32)
            nc.sync.dma_start(out=dstloc_s, in_=dstloc_in[:, :])
            rdst_s = cpool.tile([128, ntiles * H], dt.float16)
            nc.sync.dma_start(out=rdst_s, in_=rdst_in[:, :])

            # ---------- main loop over edge tiles ----------
            psb_cur = None
            eft_ld = hsT = None
            psa = None
            CH = 16  # tiles per DMA chunk (4KB per partition per stream)
            for t in range(ntiles):
                b, j = divmod(t, tpb)
                tB = t % BATCH
                tc16 = t % CH
                if tc16 == 0:
                    w = min(CH * 128, (ntiles - t) * 128)
                    eft_ld = iop.tile([128, CH * 128], dt.float16, tag="eft")
                    nc.sync.dma_start(out=eft_ld[:, 0:w],
                                      in_=eftT[:, t * 128:t * 128 + w])
                    hsT = iop.tile([128, CH * 128], dt.float16, tag="hsT")
                    nc.sync.dma_start(out=hsT[:, 0:w],
                                      in_=hsT_in[:, t * 128:t * 128 + w])
                if tB == 0:
                    psa = psEP.tile([128, BATCH, F2], dt.float32, tag="ep")

                et = eft_ld[:, tc16 * 128:(tc16 + 1) * 128]
                hst = hsT[:, tc16 * 128:(tc16 + 1) * 128]
                nc.tensor.matmul(psa[:, tB, :], lhsT=et, rhs=w2cat_s,
                                 start=True, stop=False, skip_group_check=True)
                nc.tensor.matmul(psa[:, tB, :], lhsT=hst, rhs=w1qa_s,
                                 start=False, stop=not has_bias,
                                 skip_group_check=True)
                if has_bias:
                    nc.tensor.matmul(psa[:, tB, 0:F], lhsT=ones_row, rhs=brow_s,
                                     start=False, stop=True,
                                     skip_group_check=True)

                if tB != (min(BATCH, ntiles - t + tB) - 1):
                    continue
                # ---- batch epilogue: n4 tiles of logits and messages ----
                n4 = tB + 1
                tb = t - tB
                z4 = pool.tile([128, BATCH, H], dt.float32, tag="z4")
                nc.vector.tensor_tensor(
                    out=z4[:, 0:n4, :],
                    in0=psa[:, 0:n4, F:F2],
                    in1=rdst_s[:, tb * H:(tb + n4) * H].rearrange(
                        "p (k h) -> p k h", h=H),
                    op=AOP.add)
                a4 = pool.tile([128, BATCH, H], dt.float32, tag="a4")
                nc.vector.scalar_tensor_tensor(
                    out=a4[:, 0:n4, :], in0=z4[:, 0:n4, :], scalar=0.01,
                    in1=z4[:, 0:n4, :], op0=AOP.mult, op1=AOP.max)
                msgu4 = pool.tile([128, BATCH, F2], dt.float16, tag="msgu4")
                nc.scalar.activation(
                    msgu4[:, 0:n4, F:F2], a4[:, 0:n4, :],
                    mybir.ActivationFunctionType.Exp,
                    bias=nshift[:, :])
                nc.vector.tensor_tensor(
                    out=msgu4[:, 0:n4, 0:F].rearrange("p k (h d) -> p k h d", h=H),
                    in0=psa[:, 0:n4, 0:F].rearrange("p k (h d) -> p k h d", h=H),
                    in1=msgu4[:, 0:n4, F:F2][:, :, :, None]
                        .broadcast_to((128, n4, H, DH)),
                    op=AOP.mult)

                # scatter each tile of the batch into its block accumulator
                for k in range(n4):
                    tg = tb + k
                    bb, jj = divmod(tg, tpb)
                    pk = pool.tile([128, 128], dt.float16, tag="ptile")
                    nc.vector.tensor_scalar(
                        out=pk, in0=iota_row[:, :],
                        scalar1=dstloc_s[:, tg:tg + 1], scalar2=None,
                        op0=AOP.is_equal)
                    if jj == 0:
                        psb_cur = psB.tile([128, F2], dt.float32, tag="aggB")
                    nc.tensor.matmul(psb_cur, lhsT=pk, rhs=msgu4[:, k, :],
                                     start=(jj == 0), stop=(jj == tpb - 1),
                                     skip_group_check=True)
                    if jj != tpb - 1:
                        continue
                    # ---------- phase 3 for block bb ----------
                    ss = pool.tile([128, H], dt.float32, tag="ss")
                    nc.vector.tensor_scalar(out=ss, in0=psb_cur[:, F:F2],
                                            scalar1=1e-30, scalar2=None,
                                            op0=AOP.max)
                    inv = pool.tile([128, H], dt.float32, tag="inv")
                    nc.vector.reciprocal(inv, ss)
                    mn = pool.tile([128, F], dt.float32, tag="mn")
                    nc.vector.tensor_tensor(
                        out=mn[:, :].rearrange("p (h d) -> p h d", h=H),
                        in0=psb_cur[:, 0:F].rearrange("p (h d) -> p h d", h=H),
                        in1=inv[:, :, None].broadcast_to((128, H, DH)),
                        op=AOP.mult)
                    nfs = pool.tile([128, 128], dt.float16, tag="nfs")
                    nc.sync.dma_start(out=nfs,
                                      in_=nftT_c[:, bb * 128:(bb + 1) * 128])
                    nfsm = pool.tile([128, 128], dt.float16, tag="nfsm")
                    nc.sync.dma_start(out=nfsm,
                                      in_=nftT_cm[:, bb * 128:(bb + 1) * 128])
                    psc = psC.tile([128, 128], dt.float32, tag="outC")
                    nc.tensor.matmul(psc, lhsT=w3_s, rhs=nfsm,
                                     start=True, stop=False)
                    nc.tensor.matmul(psc, lhsT=mn, rhs=ident,
                                     is_transpose=True,
                                     start=False, stop=True)
                    oc = pool.tile([128, 128], dt.float32, tag="oc")
                    nc.vector.tensor_tensor(out=oc, in0=psc, in1=nfs, op=AOP.add)
                    oc2 = pool.tile([128, 128], dt.float32, tag="oc2")
                    nc.scalar.activation(oc2, oc,
                                         mybir.ActivationFunctionType.Relu)
                    nc.sync.dma_start(out=outT[:, bb * 128:(bb + 1) * 128],
                                      in_=oc2)

    nc.compile()
    return nc


def _binpack(deg, nbins, cap):
    """LPT bin-packing of nodes into nbins bins of <= cap nodes, minimizing
    the max per-bin degree sum. Returns (bin_of_node, slot_of_node, loads)."""
    n = len(deg)
    order = np.argsort(-deg, kind="stable")
    bin_of = np.empty(n, dtype=np.int64)
    slot_of = np.empty(n, dtype=np.int64)
    counts = np.zeros(nbins, dtype=np.int64)
    loads = np.zeros(nbins, dtype=np.int64)
    heap = [(0, i) for i in range(nbins)]
    heapq.heapify(heap)
    for nd in order:
        while True:
            load, b = heapq.heappop(heap)
            if counts[b] < cap:
                break
        bin_of[nd] = b
        slot_of[nd] = counts[b]
        counts[b] += 1
        loads[b] += deg[nd]
        if counts[b] < cap:
            heapq.heappush(heap, (loads[b], b))
    return bin_of, slot_of, loads


def prep_inputs(nft, eft, W_path, b_path, W_attn1, attn2, src, dst, npc):
    """Host-side sharding/relayout. Returns (in_maps, unperm, tpb, has_bias)."""
    n_nodes = nft.shape[0]
    nb = npc // 128
    nbins = NCORES * nb

    nft = np.ascontiguousarray(nft, dtype=np.float32)
    src = np.asarray(src, dtype=np.int64)
    dst = np.asarray(dst, dtype=np.int64)
    deg = np.bincount(dst, minlength=n_nodes)

    # --- node renumbering: balance per-block edge counts ---
    bin_of, slot_of, loads = _binpack(deg, nbins, 128)
    tpb = max(1, int(np.ceil(loads.max() / 128.0)))
    # assign bins to cores (LPT on loads, 20 bins per core)
    order = np.argsort(-loads, kind="stable")
    core_of_bin = np.empty(nbins, dtype=np.int64)
    block_of_bin = np.empty(nbins, dtype=np.int64)
    cheap = [(0, c) for c in range(NCORES)]
    heapq.heapify(cheap)
    ccount = np.zeros(NCORES, dtype=np.int64)
    for g in order:
        while True:
            load, c = heapq.heappop(cheap)
            if ccount[c] < nb:
                break
        core_of_bin[g] = c
        block_of_bin[g] = ccount[c]
        ccount[c] += 1
        load += loads[g]
        if ccount[c] < nb:
            heapq.heappush(cheap, (load, c))

    ntiles = nb * tpb
    epad = ntiles * 128
    npad = nbins * 128

    nft16 = nft.astype(np.float16)

    # attention combination weights
    a2 = np.asarray(attn2, dtype=np.float32).reshape(H, DH)
    A2blk = np.zeros((F, H), dtype=np.float32)
    for h in range(H):
        A2blk[h * DH:(h + 1) * DH, h] = a2[h]
    Wp = np.ascontiguousarray(W_path, dtype=np.float32)
    W1, W2, W3 = Wp[0:F], Wp[F:2 * F], Wp[2 * F:3 * F]
    w2cat = np.concatenate([W2, W2 @ A2blk], axis=1).astype(np.float16)
    w1qa = np.concatenate(
        [W1, np.asarray(W_attn1, np.float32) + W1 @ A2blk], axis=1
    ).astype(np.float16)
    w3_np = W3.astype(np.float16)

    has_bias = bool(np.any(np.asarray(b_path) != 0))
    b = np.asarray(b_path, dtype=np.float32).reshape(F)
    bqa = b @ A2blk  # folded into rdst
    # r[dst] logit term (y3@A2): tiny per-node table, gathered on host
    r_node = (nft @ (W3 @ A2blk)).astype(np.float32)  # [N, H]

    # --- edge routing ---
    core_of_e = core_of_bin[bin_of[dst]]
    sortkey = (core_of_e * nb + block_of_bin[bin_of[dst]]) * 128 + slot_of[dst]
    perm = np.argsort(sortkey, kind="stable")
    s_key = sortkey[perm]
    s_src = src[perm]
    s_dst = dst[perm]
    s_eft = np.asarray(eft)[perm]

    in_maps = []
    blkkey = s_key // 128  # global (core*nb + block) of each sorted edge
    for c in range(NCORES):
        eftT_c = np.zeros((F, epad), dtype=np.float16)
        hsT_c = np.zeros((F, epad), dtype=np.float16)
        dstloc = np.zeros(epad, dtype=np.int64)
        rdst = np.full((epad, H), -10000.0, dtype=np.float32)

        for b_i in range(nb):
            gk = c * nb + b_i
            s = np.searchsorted(blkkey, gk)
            e = np.searchsorted(blkkey, gk + 1)
            cnt = e - s
            assert cnt <= tpb * 128, f"block overflow: {cnt} > {tpb * 128}"
            o = b_i * tpb * 128
            eftT_c[:, o:o + cnt] = s_eft[s:e].T.astype(np.float16)
            hsT_c[:, o:o + cnt] = nft16[s_src[s:e]].T
            dstloc[o:o + cnt] = slot_of[s_dst[s:e]]
            rdst[o:o + cnt] = r_node[s_dst[s:e]] + bqa

        dstloc_cols = dstloc.reshape(ntiles, 128).T.astype(np.float32).copy()
        rdst_cat = np.ascontiguousarray(
            rdst.reshape(ntiles, 128, H).transpose(1, 0, 2).reshape(128, ntiles * H)
        ).astype(np.float16)

        # per-core node features (new order)
        ids = np.arange(npc, dtype=np.int64)
        gbin = c * nb + ids // 128
        rows = np.zeros((npc, F), dtype=np.float32)
        mask = np.zeros(npc, dtype=bool)
        # nodes whose (bin) is assigned to this core at block ids//128
        for b_i in range(nb):
            # find global bin g with core_of_bin[g]==c and block_of_bin[g]==b_i
            g = np.where((core_of_bin == c) & (block_of_bin == b_i))[0][0]
            sel = bin_of == g
            nds = np.where(sel)[0]
            sl = slot_of[nds]
            rows[b_i * 128 + sl] = nft[nds]
            mask[b_i * 128 + sl] = deg[nds] > 0
        nftT_c = rows.T.astype(np.float16).copy()
        nftT_cm = (rows * mask[:, None]).T.astype(np.float16).copy()

        m = {
            "eftT": eftT_c,
            "hsT": hsT_c,
            "nftT_c": nftT_c,
            "nftT_cm": nftT_cm,
            "w2cat": w2cat,
            "w1qa": w1qa,
            "w3": w3_np,
            "dstloc": dstloc_cols,
            "rdst": rdst_cat,
        }
        if has_bias:
            m["brow"] = b.astype(np.float16).reshape(1, F)
        in_maps.append(m)

    unperm = (core_of_bin, block_of_bin, bin_of, slot_of)
    return in_maps, unperm, tpb, has_bias


_NC_CACHE = {}


def _get_nc(key, *args, **kw):
    if key not in _NC_CACHE:
        _NC_CACHE[key] = build_nc(*args, **kw)
    return _NC_CACHE[key]


def run(nft, eft, W_path, b_path, W_attn1, attn2, src, dst, trace=False,
        tmpdir=None, prec="f16"):
    n_nodes = nft.shape[0]
    npc = ((n_nodes + NCORES - 1) // NCORES + 127) // 128 * 128
    nb = npc // 128

    in_maps, unperm, tpb, has_bias = prep_inputs(
        np.asarray(nft), np.asarray(eft), np.asarray(W_path),
        np.asarray(b_path), np.asarray(W_attn1), np.asarray(attn2),
        np.asarray(src), np.asarray(dst), npc)
    core_of_bin, block_of_bin, bin_of, slot_of = unperm
    npad = NCORES * nb * 128

    nc = _get_nc((npc, tpb, npad, has_bias), npc, tpb, npad, has_bias)
    kw = {}
    if trace:
        kw = dict(trace=True, tmpdir=tmpdir)
    res = bass_utils.run_bass_kernel_spmd(nc, in_maps,
                                          core_ids=list(range(NCORES)), **kw)

    out = np.empty((n_nodes, F), dtype=np.float32)
    cols = block_of_bin[bin_of] * 128 + slot_of  # column in owning core's outT
    cores = core_of_bin[bin_of]
    for c in range(NCORES):
        sel = cores == c
        out[sel] = res.results[c]["outT"][:, cols[sel]].T
    return out, res


def kernel(**inputs):
    out, _ = run(**inputs)
    return out


# revision 23
# speedup vs baseline: 1.2448x; 1.2448x over previous
"""GAT message-passing kernel for 8 Trainium2 NeuronCores (Bass/Tile).

Strategy ("route edges by dst ownership", no collectives):
  - Host renumbers nodes (LPT bin-packing by degree) into 160 blocks of <=128
    nodes so every 128-node block has nearly equal in-edge count; each core
    owns 20 blocks, so segment-softmax and scatter-sum are fully core-local.
  - Reassociation: epaths = y1[src] + eft@W2 + y3[dst] + b with y1 = nft@W1.
    Since sum(att)=1 per (node, head), the y3[dst] part of the aggregated
    message is exactly +y3[dst], added once per node in phase 3.  Softmax is
    computed without max-subtraction; a fixed shift exp(a-7) keeps the
    unnormalized weights in fp16 range (softmax is shift-invariant).
  - No node-table phase and no device-side gather: the src node features are
    gathered per edge on the HOST (the sharding hint: "each device holds the
    gathered src/dst features") into a dense transposed tensor hsT that the
    kernel streams with full-rate 2KB DMA descriptors, and y1/qa are computed
    per edge on the PE:
      psEP[:, k, 0:136] = eftT_tile.T @ [W2 | W2A2] + hsT_tile.T @ [W1 | Wqa]
    where Wqa = W_attn1 + W1@A2. The r[dst] = y3@A2 logit term is tiny
    (8 floats per node) and is gathered per edge on the host (rdst); dummy
    padding edges get rdst = -1e4 which drives their weight u to exp(<-100)=0.
  - Per tile: one-hot P (dst-slot x edge) is built on-chip by a DVE is_equal
    against an iota row; P is the stationary of the scatter matmul that
    accumulates [agg | s] for the owning 128-node block.
  - Phase 3 (per node block): agg/s, transpose via PE, += nft@W3 (matmul
    accumulate, deg>0-masked) and += nft, relu, store transposed output.
"""

import sys
import heapq
import numpy as np

for _p in ("/opt/trn_rl_repo",):
    if _p not in sys.path:
        sys.path.append(_p)

import concourse.bacc as bacc
import concourse.bass as bass
import concourse.mybir as mybir
from concourse.tile import TileContext
from concourse import bass_utils

F = 128
H = 8
DH = 16
F2 = F + H  # 136
NCORES = 8
EXP_SHIFT = 7.0  # exp(a - shift); softmax-invariant, keeps u in fp16 range
BATCH = 3        # tiles per PSUM epilogue batch ([128, 3, 136] f32 <= 2KB bank)


def build_nc(npc, tpb, npad, has_bias, debug=False):
    nb = npc // 128                  # node blocks per core
    ntiles = nb * tpb                # edge tiles per core
    epad = ntiles * 128              # padded edge count per core
    dt = mybir.dt
    AOP = mybir.AluOpType

    nc = bacc.Bacc("TRN2", target_bir_lowering=False, debug=False,
                   num_devices=NCORES, num_swdge_queues=4)

    # ---- inputs ----
    eftT = nc.dram_tensor("eftT", (F, epad), dt.float16, kind="ExternalInput")
    hsT_in = nc.dram_tensor("hsT", (F, epad), dt.float16, kind="ExternalInput")
    nftT_c = nc.dram_tensor("nftT_c", (F, npc), dt.float16, kind="ExternalInput")
    nftT_cm = nc.dram_tensor("nftT_cm", (F, npc), dt.float16, kind="ExternalInput")
    w2cat_in = nc.dram_tensor("w2cat", (F, F2), dt.float16, kind="ExternalInput")
    w1qa_in = nc.dram_tensor("w1qa", (F, F2), dt.float16, kind="ExternalInput")
    w3_in = nc.dram_tensor("w3", (F, F), dt.float16, kind="ExternalInput")
    dstloc_in = nc.dram_tensor("dstloc", (128, ntiles), dt.float16, kind="ExternalInput")
    rdst_in = nc.dram_tensor("rdst", (128, ntiles * H), dt.float16, kind="ExternalInput")
    if has_bias:
        brow_in = nc.dram_tensor("brow", (1, F), dt.float16, kind="ExternalInput")

    outT = nc.dram_tensor("outT", (F, npc), dt.float32, kind="ExternalOutput")

    with TileContext(nc) as tc:
        with tc.tile_pool(name="const", bufs=1) as cpool, \
             tc.tile_pool(name="work", bufs=3) as pool, \
             tc.tile_pool(name="io", bufs=4) as iop, \
             tc.tile_pool(name="psEP", bufs=3, space="PSUM") as psEP, \
             tc.tile_pool(name="psB", bufs=2, space="PSUM") as psB, \
             tc.tile_pool(name="psC", bufs=2, space="PSUM") as psC:

            # ---------- constants ----------
            iota_row = cpool.tile([128, 128], dt.float32)
            nc.gpsimd.iota(iota_row, pattern=[[1, 128]], channel_multiplier=0,
                           allow_small_or_imprecise_dtypes=True)
            iota_col = cpool.tile([128, 1], dt.float32)
            nc.gpsimd.iota(iota_col, pattern=[[1, 1]], channel_multiplier=1,
                           allow_small_or_imprecise_dtypes=True)
            ident = cpool.tile([128, 128], dt.float32)
            nc.vector.tensor_scalar(out=ident, in0=iota_row[:, :],
                                    scalar1=iota_col[:, :], scalar2=None,
                                    op0=AOP.is_equal)
            nshift = cpool.tile([128, 1], dt.float32)
            nc.vector.memset(nshift, -EXP_SHIFT)
            iota16 = cpool.tile([128, 128], dt.float16)
            nc.vector.tensor_copy(out=iota16, in_=iota_row)

            w2cat_s = cpool.tile([F, F2], dt.float16)
            nc.sync.dma_start(out=w2cat_s, in_=w2cat_in[:, :])
            w1qa_s = cpool.tile([F, F2], dt.float16)
            nc.sync.dma_start(out=w1qa_s, in_=w1qa_in[:, :])
            w3_s = cpool.tile([F, F], dt.float16)
            nc.sync.dma_start(out=w3_s, in_=w3_in[:, :])

            if has_bias:
                brow_s = cpool.tile([1, F], dt.float16)
                nc.sync.dma_start(out=brow_s, in_=brow_in[:, :])
                ones_row = cpool.tile([1, 128], dt.float16)
                nc.vector.memset(ones_row, 1.0)

            dstloc_s = cpool.tile([128, ntiles], dt.float16)
            nc.sync.dma_start(out=dstloc_s, in_=dstloc_in[:, :])
            rdst_s = cpool.tile([128, ntiles * H], dt.float16)
            nc.sync.dma_start(out=rdst_s, in_=rdst_in[:, :])

            # ---------- main loop over edge tiles ----------
            psb_cur = None
            eft_ld = hsT = None
            psa = None
            CH = 16  # tiles per DMA chunk (4KB per partition per stream)
            pch_hist = {}
            for t in range(ntiles):
                b, j = divmod(t, tpb)
                tB = t % BATCH
                tc16 = t % CH
                if tc16 == 0:
                    w = min(CH * 128, (ntiles - t) * 128)
                    nch16 = w // 128
                    eft_ld = iop.tile([128, CH * 128], dt.float16, tag="eft")
                    nc.sync.dma_start(out=eft_ld[:, 0:w],
                                      in_=eftT[:, t * 128:t * 128 + w])
                    hsT = iop.tile([128, CH * 128], dt.float16, tag="hsT")
                    nc.sync.dma_start(out=hsT[:, 0:w],
                                      in_=hsT_in[:, t * 128:t * 128 + w])
                    # one-hot P for the whole chunk in one DVE op
                    pchunk = iop.tile([128, CH, 128], dt.float16, tag="pchunk")
                    nc.vector.tensor_tensor(
                        out=pchunk[:, 0:nch16, :],
                        in0=dstloc_s[:, t:t + nch16, None]
                            .broadcast_to((128, nch16, 128)),
                        in1=iota16[:, None, :].broadcast_to((128, nch16, 128)),
                        op=AOP.is_equal)
                    pch_hist[t // CH] = pchunk
                    pch_hist.pop(t // CH - 2, None)
                if tB == 0:
                    psa = psEP.tile([128, BATCH, F2], dt.float32, tag="ep")

                et = eft_ld[:, tc16 * 128:(tc16 + 1) * 128]
                hst = hsT[:, tc16 * 128:(tc16 + 1) * 128]
                nc.tensor.matmul(psa[:, tB, :], lhsT=et, rhs=w2cat_s,
                                 start=True, stop=False, skip_group_check=True)
                nc.tensor.matmul(psa[:, tB, :], lhsT=hst, rhs=w1qa_s,
                                 start=False, stop=not has_bias,
                                 skip_group_check=True)
                if has_bias:
                    nc.tensor.matmul(psa[:, tB, 0:F], lhsT=ones_row, rhs=brow_s,
                                     start=False, stop=True,
                                     skip_group_check=True)

                if tB != (min(BATCH, ntiles - t + tB) - 1):
                    continue
                # ---- batch epilogue: n4 tiles of logits and messages ----
                n4 = tB + 1
                tb = t - tB
                z4 = pool.tile([128, BATCH, H], dt.float32, tag="z4")
                nc.vector.tensor_tensor(
                    out=z4[:, 0:n4, :],
                    in0=psa[:, 0:n4, F:F2],
                    in1=rdst_s[:, tb * H:(tb + n4) * H].rearrange(
                        "p (k h) -> p k h", h=H),
                    op=AOP.add)
                a4 = pool.tile([128, BATCH, H], dt.float32, tag="a4")
                nc.vector.scalar_tensor_tensor(
                    out=a4[:, 0:n4, :], in0=z4[:, 0:n4, :], scalar=0.01,
                    in1=z4[:, 0:n4, :], op0=AOP.mult, op1=AOP.max)
                msgu4 = pool.tile([128, BATCH, F2], dt.float16, tag="msgu4")
                nc.scalar.activation(
                    msgu4[:, 0:n4, F:F2], a4[:, 0:n4, :],
                    mybir.ActivationFunctionType.Exp,
                    bias=nshift[:, :])
                nc.vector.tensor_tensor(
                    out=msgu4[:, 0:n4, 0:F].rearrange("p k (h d) -> p k h d", h=H),
                    in0=psa[:, 0:n4, 0:F].rearrange("p k (h d) -> p k h d", h=H),
                    in1=msgu4[:, 0:n4, F:F2][:, :, :, None]
                        .broadcast_to((128, n4, H, DH)),
                    op=AOP.mult)

                # scatter each tile of the batch into its block accumulator
                for k in range(n4):
                    tg = tb + k
                    bb, jj = divmod(tg, tpb)
                    pk = pch_hist[tg // CH][:, tg % CH, :]
                    if jj == 0:
                        psb_cur = psB.tile([128, F2], dt.float32, tag="aggB")
                    nc.tensor.matmul(psb_cur, lhsT=pk, rhs=msgu4[:, k, :],
                                     start=(jj == 0), stop=(jj == tpb - 1),
                                     skip_group_check=True)
                    if jj != tpb - 1:
                        continue
                    # ---------- phase 3 for block bb ----------
                    ss = pool.tile([128, H], dt.float32, tag="ss")
                    nc.vector.tensor_scalar(out=ss, in0=psb_cur[:, F:F2],
                                            scalar1=1e-30, scalar2=None,
                                            op0=AOP.max)
                    inv = pool.tile([128, H], dt.float32, tag="inv")
                    nc.vector.reciprocal(inv, ss)
                    mn = pool.tile([128, F], dt.float32, tag="mn")
                    nc.vector.tensor_tensor(
                        out=mn[:, :].rearrange("p (h d) -> p h d", h=H),
                        in0=psb_cur[:, 0:F].rearrange("p (h d) -> p h d", h=H),
                        in1=inv[:, :, None].broadcast_to((128, H, DH)),
                        op=AOP.mult)
                    nfs = pool.tile([128, 128], dt.float16, tag="nfs")
                    nc.sync.dma_start(out=nfs,
                                      in_=nftT_c[:, bb * 128:(bb + 1) * 128])
                    nfsm = pool.tile([128, 128], dt.float16, tag="nfsm")
                    nc.sync.dma_start(out=nfsm,
                                      in_=nftT_cm[:, bb * 128:(bb + 1) * 128])
                    psc = psC.tile([128, 128], dt.float32, tag="outC")
                    nc.tensor.matmul(psc, lhsT=w3_s, rhs=nfsm,
                                     start=True, stop=False)
                    nc.tensor.matmul(psc, lhsT=mn, rhs=ident,
                                     is_transpose=True,
                                     start=False, stop=True)
                    oc = pool.tile([128, 128], dt.float32, tag="oc")
                    nc.vector.tensor_tensor(out=oc, in0=psc, in1=nfs, op=AOP.add)
                    oc2 = pool.tile([128, 128], dt.float32, tag="oc2")
                    nc.scalar.activation(oc2, oc,
                                         mybir.ActivationFunctionType.Relu)
                    nc.sync.dma_start(out=outT[:, bb * 128:(bb + 1) * 128],
                                      in_=oc2)

    nc.compile()
    return nc


def _binpack(deg, nbins, cap):
    """LPT bin-packing of nodes into nbins bins of <= cap nodes, minimizing
    the max per-bin degree sum. Returns (bin_of_node, slot_of_node, loads)."""
    n = len(deg)
    order = np.argsort(-deg, kind="stable")
    bin_of = np.empty(n, dtype=np.int64)
    slot_of = np.empty(n, dtype=np.int64)
    counts = np.zeros(nbins, dtype=np.int64)
    loads = np.zeros(nbins, dtype=np.int64)
    heap = [(0, i) for i in range(nbins)]
    heapq.heapify(heap)
    for nd in order:
        while True:
            load, b = heapq.heappop(heap)
            if counts[b] < cap:
                break
        bin_of[nd] = b
        slot_of[nd] = counts[b]
        counts[b] += 1
        loads[b] += deg[nd]
        if counts[b] < cap:
            heapq.heappush(heap, (loads[b], b))
    return bin_of, slot_of, loads


def prep_inputs(nft, eft, W_path, b_path, W_attn1, attn2, src, dst, npc):
    """Host-side sharding/relayout. Returns (in_maps, unperm, tpb, has_bias)."""
    n_nodes = nft.shape[0]
    nb = npc // 128
    nbins = NCORES * nb

    nft = np.ascontiguousarray(nft, dtype=np.float32)
    src = np.asarray(src, dtype=np.int64)
    dst = np.asarray(dst, dtype=np.int64)
    deg = np.bincount(dst, minlength=n_nodes)

    # --- node renumbering: balance per-block edge counts ---
    bin_of, slot_of, loads = _binpack(deg, nbins, 128)
    tpb = max(1, int(np.ceil(loads.max() / 128.0)))
    # assign bins to cores (LPT on loads, 20 bins per core)
    order = np.argsort(-loads, kind="stable")
    core_of_bin = np.empty(nbins, dtype=np.int64)
    block_of_bin = np.empty(nbins, dtype=np.int64)
    cheap = [(0, c) for c in range(NCORES)]
    heapq.heapify(cheap)
    ccount = np.zeros(NCORES, dtype=np.int64)
    for g in order:
        while True:
            load, c = heapq.heappop(cheap)
            if ccount[c] < nb:
                break
        core_of_bin[g] = c
        block_of_bin[g] = ccount[c]
        ccount[c] += 1
        load += loads[g]
        if ccount[c] < nb:
            heapq.heappush(cheap, (load, c))

    ntiles = nb * tpb
    epad = ntiles * 128
    npad = nbins * 128

    nft16 = nft.astype(np.float16)

    # attention combination weights
    a2 = np.asarray(attn2, dtype=np.float32).reshape(H, DH)
    A2blk = np.zeros((F, H), dtype=np.float32)
    for h in range(H):
        A2blk[h * DH:(h + 1) * DH, h] = a2[h]
    Wp = np.ascontiguousarray(W_path, dtype=np.float32)
    W1, W2, W3 = Wp[0:F], Wp[F:2 * F], Wp[2 * F:3 * F]
    w2cat = np.concatenate([W2, W2 @ A2blk], axis=1).astype(np.float16)
    w1qa = np.concatenate(
        [W1, np.asarray(W_attn1, np.float32) + W1 @ A2blk], axis=1
    ).astype(np.float16)
    w3_np = W3.astype(np.float16)

    has_bias = bool(np.any(np.asarray(b_path) != 0))
    b = np.asarray(b_path, dtype=np.float32).reshape(F)
    bqa = b @ A2blk  # folded into rdst
    # r[dst] logit term (y3@A2): tiny per-node table, gathered on host
    r_node = (nft @ (W3 @ A2blk)).astype(np.float32)  # [N, H]

    # --- edge routing ---
    core_of_e = core_of_bin[bin_of[dst]]
    sortkey = (core_of_e * nb + block_of_bin[bin_of[dst]]) * 128 + slot_of[dst]
    perm = np.argsort(sortkey, kind="stable")
    s_key = sortkey[perm]
    s_src = src[perm]
    s_dst = dst[perm]
    s_eft = np.asarray(eft)[perm]

    in_maps = []
    blkkey = s_key // 128  # global (core*nb + block) of each sorted edge
    for c in range(NCORES):
        eftT_c = np.zeros((F, epad), dtype=np.float16)
        hsT_c = np.zeros((F, epad), dtype=np.float16)
        dstloc = np.zeros(epad, dtype=np.int64)
        rdst = np.full((epad, H), -10000.0, dtype=np.float32)

        for b_i in range(nb):
            gk = c * nb + b_i
            s = np.searchsorted(blkkey, gk)
            e = np.searchsorted(blkkey, gk + 1)
            cnt = e - s
            assert cnt <= tpb * 128, f"block overflow: {cnt} > {tpb * 128}"
            o = b_i * tpb * 128
            eftT_c[:, o:o + cnt] = s_eft[s:e].T.astype(np.float16)
            hsT_c[:, o:o + cnt] = nft16[s_src[s:e]].T
            dstloc[o:o + cnt] = slot_of[s_dst[s:e]]
            rdst[o:o + cnt] = r_node[s_dst[s:e]] + bqa

        dstloc_cols = dstloc.reshape(ntiles, 128).T.astype(np.float16).copy()
        rdst_cat = np.ascontiguousarray(
            rdst.reshape(ntiles, 128, H).transpose(1, 0, 2).reshape(128, ntiles * H)
        ).astype(np.float16)

        # per-core node features (new order)
        ids = np.arange(npc, dtype=np.int64)
        gbin = c * nb + ids // 128
        rows = np.zeros((npc, F), dtype=np.float32)
        mask = np.zeros(npc, dtype=bool)
        # nodes whose (bin) is assigned to this core at block ids//128
        for b_i in range(nb):
            # find global bin g with core_of_bin[g]==c and block_of_bin[g]==b_i
            g = np.where((core_of_bin == c) & (block_of_bin == b_i))[0][0]
            sel = bin_of == g
            nds = np.where(sel)[0]
            sl = slot_of[nds]
            rows[b_i * 128 + sl] = nft[nds]
            mask[b_i * 128 + sl] = deg[nds] > 0
        nftT_c = rows.T.astype(np.float16).copy()
        nftT_cm = (rows * mask[:, None]).T.astype(np.float16).copy()

        m = {
            "eftT": eftT_c,
            "hsT": hsT_c,
            "nftT_c": nftT_c,
            "nftT_cm": nftT_cm,
            "w2cat": w2cat,
            "w1qa": w1qa,
            "w3": w3_np,
            "dstloc": dstloc_cols,
            "rdst": rdst_cat,
        }
        if has_bias:
            m["brow"] = b.astype(np.float16).reshape(1, F)
        in_maps.append(m)

    unperm = (core_of_bin, block_of_bin, bin_of, slot_of)
    return in_maps, unperm, tpb, has_bias


_NC_CACHE = {}


def _get_nc(key, *args, **kw):
    if key not in _NC_CACHE:
        _NC_CACHE[key] = build_nc(*args, **kw)
    return _NC_CACHE[key]


def run(nft, eft, W_path, b_path, W_attn1, attn2, src, dst, trace=False,
        tmpdir=None, prec="f16"):
    n_nodes = nft.shape[0]
    npc = ((n_nodes + NCORES - 1) // NCORES + 127) // 128 * 128
    nb = npc // 128

    in_maps, unperm, tpb, has_bias = prep_inputs(
        np.asarray(nft), np.asarray(eft), np.asarray(W_path),
        np.asarray(b_path), np.asarray(W_attn1), np.asarray(attn2),
        np.asarray(src), np.asarray(dst), npc)
    core_of_bin, block_of_bin, bin_of, slot_of = unperm
    npad = NCORES * nb * 128

    nc = _get_nc((npc, tpb, npad, has_bias), npc, tpb, npad, has_bias)
    kw = {}
    if trace:
        kw = dict(trace=True, tmpdir=tmpdir)
    res = bass_utils.run_bass_kernel_spmd(nc, in_maps,
                                          core_ids=list(range(NCORES)), **kw)

    out = np.empty((n_nodes, F), dtype=np.float32)
    cols = block_of_bin[bin_of] * 128 + slot_of  # column in owning core's outT
    cores = core_of_bin[bin_of]
    for c in range(NCORES):
        sel = cores == c
        out[sel] = res.results[c]["outT"][:, cols[sel]].T
    return out, res


def kernel(**inputs):
    out, _ = run(**inputs)
    return out


# revision 31
# speedup vs baseline: 1.2592x; 1.0116x over previous
"""GAT message-passing kernel for 8 Trainium2 NeuronCores (Bass/Tile).

Strategy ("route edges by dst ownership", no collectives):
  - Host renumbers nodes (LPT bin-packing by degree) into 160 blocks of <=128
    nodes so every 128-node block has nearly equal in-edge count; each core
    owns 20 blocks, so segment-softmax and scatter-sum are fully core-local.
  - Reassociation: epaths = y1[src] + eft@W2 + y3[dst] + b with y1 = nft@W1.
    Since sum(att)=1 per (node, head), the y3[dst] part of the aggregated
    message is exactly +y3[dst], added once per node in phase 3.  Softmax is
    computed without max-subtraction; a fixed shift exp(a-7) keeps the
    unnormalized weights in fp16 range (softmax is shift-invariant).
  - No node-table phase and no device-side gather: the src node features are
    gathered per edge on the HOST (the sharding hint: "each device holds the
    gathered src/dst features") into a dense transposed tensor hsT that the
    kernel streams with full-rate 2KB DMA descriptors, and y1/qa are computed
    per edge on the PE:
      psEP[:, k, 0:136] = eftT_tile.T @ [W2 | W2A2] + hsT_tile.T @ [W1 | Wqa]
    where Wqa = W_attn1 + W1@A2. The r[dst] = y3@A2 logit term is tiny
    (8 floats per node) and is gathered per edge on the host (rdst); dummy
    padding edges get rdst = -1e4 which drives their weight u to exp(<-100)=0.
  - Per tile: one-hot P (dst-slot x edge) is built on-chip by a DVE is_equal
    against an iota row; P is the stationary of the scatter matmul that
    accumulates [agg | s] for the owning 128-node block.
  - Phase 3 (per node block): agg/s, transpose via PE, += nft@W3 (matmul
    accumulate, deg>0-masked) and += nft, relu, store transposed output.
"""

import sys
import heapq
import numpy as np

for _p in ("/opt/trn_rl_repo",):
    if _p not in sys.path:
        sys.path.append(_p)

import concourse.bacc as bacc
import concourse.bass as bass
import concourse.mybir as mybir
from concourse.tile import TileContext
from concourse import bass_utils

F = 128
H = 8
DH = 16
F2 = F + H  # 136
NCORES = 8
EXP_SHIFT = 7.0  # exp(a - shift); softmax-invariant, keeps u in fp16 range
BATCH = 3        # tiles per PSUM epilogue batch ([128, 3, 136] f32 <= 2KB bank)


def build_nc(npc, tpb, npad, has_bias, debug=False):
    nb = npc // 128                  # node blocks per core
    ntiles = nb * tpb                # edge tiles per core
    epad = ntiles * 128              # padded edge count per core
    dt = mybir.dt
    AOP = mybir.AluOpType

    nc = bacc.Bacc("TRN2", target_bir_lowering=False, debug=False,
                   num_devices=NCORES, num_swdge_queues=4)

    # ---- inputs ----
    eftT = nc.dram_tensor("eftT", (F, epad), dt.float16, kind="ExternalInput")
    hsT_in = nc.dram_tensor("hsT", (F, epad), dt.float16, kind="ExternalInput")
    nftT_c = nc.dram_tensor("nftT_c", (F, npc), dt.float16, kind="ExternalInput")
    nftT_cm = nc.dram_tensor("nftT_cm", (F, npc), dt.float16, kind="ExternalInput")
    w2cat_in = nc.dram_tensor("w2cat", (F, F2), dt.float16, kind="ExternalInput")
    w1qa_in = nc.dram_tensor("w1qa", (F, F2), dt.float16, kind="ExternalInput")
    w3_in = nc.dram_tensor("w3", (F, F), dt.float16, kind="ExternalInput")
    dstloc_in = nc.dram_tensor("dstloc", (128, ntiles), dt.float16, kind="ExternalInput")
    rdst_in = nc.dram_tensor("rdst", (128, ntiles * H), dt.float16, kind="ExternalInput")
    if has_bias:
        brow_in = nc.dram_tensor("brow", (1, F), dt.float16, kind="ExternalInput")

    outT = nc.dram_tensor("outT", (F, npc), dt.float32, kind="ExternalOutput")

    with TileContext(nc) as tc:
        with tc.tile_pool(name="const", bufs=1) as cpool, \
             tc.tile_pool(name="work", bufs=3) as pool, \
             tc.tile_pool(name="io", bufs=4) as iop, \
             tc.tile_pool(name="psEP", bufs=3, space="PSUM") as psEP, \
             tc.tile_pool(name="psB", bufs=2, space="PSUM") as psB, \
             tc.tile_pool(name="psC", bufs=2, space="PSUM") as psC:

            # ---------- constants ----------
            iota_row = cpool.tile([128, 128], dt.float32)
            nc.gpsimd.iota(iota_row, pattern=[[1, 128]], channel_multiplier=0,
                           allow_small_or_imprecise_dtypes=True)
            iota_col = cpool.tile([128, 1], dt.float32)
            nc.gpsimd.iota(iota_col, pattern=[[1, 1]], channel_multiplier=1,
                           allow_small_or_imprecise_dtypes=True)
            ident = cpool.tile([128, 128], dt.float32)
            nc.vector.tensor_scalar(out=ident, in0=iota_row[:, :],
                                    scalar1=iota_col[:, :], scalar2=None,
                                    op0=AOP.is_equal)
            nshift = cpool.tile([128, 1], dt.float32)
            nc.vector.memset(nshift, -EXP_SHIFT)
            iota16 = cpool.tile([128, 128], dt.float16)
            nc.vector.tensor_copy(out=iota16, in_=iota_row)
            # 0..127 repeated per chunk tile, unit-stride (for one-hot build)
            iota_big = cpool.tile([128, 32 * 128], dt.float16)
            nc.gpsimd.iota(iota_big, pattern=[[0, 32], [1, 128]],
                           channel_multiplier=0,
                           allow_small_or_imprecise_dtypes=True)

            w2cat_s = cpool.tile([F, F2], dt.float16)
            nc.sync.dma_start(out=w2cat_s, in_=w2cat_in[:, :])
            w1qa_s = cpool.tile([F, F2], dt.float16)
            nc.sync.dma_start(out=w1qa_s, in_=w1qa_in[:, :])
            w3_s = cpool.tile([F, F], dt.float16)
            nc.sync.dma_start(out=w3_s, in_=w3_in[:, :])

            if has_bias:
                brow_s = cpool.tile([1, F], dt.float16)
                nc.sync.dma_start(out=brow_s, in_=brow_in[:, :])
                ones_row = cpool.tile([1, 128], dt.float16)
                nc.vector.memset(ones_row, 1.0)

            dstloc_s = cpool.tile([128, ntiles], dt.float16)
            nc.sync.dma_start(out=dstloc_s, in_=dstloc_in[:, :])
            rdst_s = cpool.tile([128, ntiles * H], dt.float16)
            nc.sync.dma_start(out=rdst_s, in_=rdst_in[:, :])
            nftc_s = cpool.tile([128, npc], dt.float16)
            nc.sync.dma_start(out=nftc_s, in_=nftT_c[:, :])
            nftcm_s = cpool.tile([128, npc], dt.float16)
            nc.sync.dma_start(out=nftcm_s, in_=nftT_cm[:, :])

            # ---------- main loop over edge tiles ----------
            psb_cur = None
            eft_ld = hsT = None
            psa = None
            CH = 32  # tiles per DMA chunk (8KB per partition per stream)
            pch_hist = {}
            for t in range(ntiles):
                b, j = divmod(t, tpb)
                tB = t % BATCH
                tc16 = t % CH
                if tc16 == 0:
                    w = min(CH * 128, (ntiles - t) * 128)
                    nch16 = w // 128
                    eft_ld = iop.tile([128, CH * 128], dt.float16, tag="eft")
                    nc.sync.dma_start(out=eft_ld[:, 0:w],
                                      in_=eftT[:, t * 128:t * 128 + w])
                    hsT = iop.tile([128, CH * 128], dt.float16, tag="hsT")
                    nc.sync.dma_start(out=hsT[:, 0:w],
                                      in_=hsT_in[:, t * 128:t * 128 + w])
                    # one-hot P for the whole chunk in one DVE op
                    pchunk = iop.tile([128, CH, 128], dt.float16, tag="pchunk")
                    nc.vector.tensor_tensor(
                        out=pchunk[:, 0:nch16, :],
                        in0=dstloc_s[:, t:t + nch16, None]
                            .broadcast_to((128, nch16, 128)),
                        in1=iota_big[:, 0:nch16 * 128].rearrange(
                            "p (k c) -> p k c", c=128),
                        op=AOP.is_equal)
                    pch_hist[t // CH] = pchunk
                    pch_hist.pop(t // CH - 2, None)
                if tB == 0:
                    psa = psEP.tile([128, BATCH, F2], dt.float32, tag="ep")

                et = eft_ld[:, tc16 * 128:(tc16 + 1) * 128]
                hst = hsT[:, tc16 * 128:(tc16 + 1) * 128]
                nc.tensor.matmul(psa[:, tB, :], lhsT=et, rhs=w2cat_s,
                                 start=True, stop=False, skip_group_check=True)
                nc.tensor.matmul(psa[:, tB, :], lhsT=hst, rhs=w1qa_s,
                                 start=False, stop=not has_bias,
                                 skip_group_check=True)
                if has_bias:
                    nc.tensor.matmul(psa[:, tB, 0:F], lhsT=ones_row, rhs=brow_s,
                                     start=False, stop=True,
                                     skip_group_check=True)

                if tB != (min(BATCH, ntiles - t + tB) - 1):
                    continue
                # ---- batch epilogue: n4 tiles of logits and messages ----
                n4 = tB + 1
                tb = t - tB
                z4 = pool.tile([128, BATCH, H], dt.float32, tag="z4")
                nc.vector.tensor_tensor(
                    out=z4[:, 0:n4, :],
                    in0=psa[:, 0:n4, F:F2],
                    in1=rdst_s[:, tb * H:(tb + n4) * H].rearrange(
                        "p (k h) -> p k h", h=H),
                    op=AOP.add)
                a4 = pool.tile([128, BATCH, H], dt.float32, tag="a4")
                nc.vector.scalar_tensor_tensor(
                    out=a4[:, 0:n4, :], in0=z4[:, 0:n4, :], scalar=0.01,
                    in1=z4[:, 0:n4, :], op0=AOP.mult, op1=AOP.max)
                msgu4 = pool.tile([128, BATCH, F2], dt.float16, tag="msgu4")
                nc.scalar.activation(
                    msgu4[:, 0:n4, F:F2], a4[:, 0:n4, :],
                    mybir.ActivationFunctionType.Exp,
                    bias=nshift[:, :])
                # u expanded 16x along features (unit-stride for the mult)
                uexp = pool.tile([128, BATCH, F], dt.float16, tag="uexp")
                nc.scalar.activation(
                    uexp[:, 0:n4, :].rearrange("p k (h d) -> p k h d", h=H),
                    a4[:, 0:n4, :, None].broadcast_to((128, n4, H, DH)),
                    mybir.ActivationFunctionType.Exp,
                    bias=nshift[:, :])
                nc.vector.tensor_tensor(
                    out=msgu4[:, 0:n4, 0:F],
                    in0=psa[:, 0:n4, 0:F],
                    in1=uexp[:, 0:n4, :],
                    op=AOP.mult)

                # scatter each tile of the batch into its block accumulator
                for k in range(n4):
                    tg = tb + k
                    bb, jj = divmod(tg, tpb)
                    pk = pch_hist[tg // CH][:, tg % CH, :]
                    if jj == 0:
                        psb_cur = psB.tile([128, F2], dt.float32, tag="aggB")
                    nc.tensor.matmul(psb_cur, lhsT=pk, rhs=msgu4[:, k, :],
                                     start=(jj == 0), stop=(jj == tpb - 1),
                                     skip_group_check=True)
                    if jj != tpb - 1:
                        continue
                    # ---------- phase 3 for block bb ----------
                    ss = pool.tile([128, H], dt.float32, tag="ss")
                    nc.vector.tensor_scalar(out=ss, in0=psb_cur[:, F:F2],
                                            scalar1=1e-30, scalar2=None,
                                            op0=AOP.max)
                    inv = pool.tile([128, H], dt.float32, tag="inv")
                    nc.vector.reciprocal(inv, ss)
                    mn = pool.tile([128, F], dt.float32, tag="mn")
                    nc.vector.tensor_tensor(
                        out=mn[:, :].rearrange("p (h d) -> p h d", h=H),
                        in0=psb_cur[:, 0:F].rearrange("p (h d) -> p h d", h=H),
                        in1=inv[:, :, None].broadcast_to((128, H, DH)),
                        op=AOP.mult)
                    psc = psC.tile([128, 128], dt.float32, tag="outC")
                    nc.tensor.matmul(psc, lhsT=w3_s,
                                     rhs=nftcm_s[:, bb * 128:(bb + 1) * 128],
                                     start=True, stop=False)
                    nc.tensor.matmul(psc, lhsT=mn, rhs=ident,
                                     is_transpose=True,
                                     start=False, stop=True)
                    oc = pool.tile([128, 128], dt.float32, tag="oc")
                    nc.vector.tensor_tensor(
                        out=oc, in0=psc,
                        in1=nftc_s[:, bb * 128:(bb + 1) * 128], op=AOP.add)
                    oc2 = pool.tile([128, 128], dt.float32, tag="oc2")
                    nc.scalar.activation(oc2, oc,
                                         mybir.ActivationFunctionType.Relu)
                    nc.sync.dma_start(out=outT[:, bb * 128:(bb + 1) * 128],
                                      in_=oc2)

    nc.compile()
    return nc


def _binpack(deg, nbins, cap):
    """LPT bin-packing of nodes into nbins bins of <= cap nodes, minimizing
    the max per-bin degree sum. Returns (bin_of_node, slot_of_node, loads)."""
    n = len(deg)
    order = np.argsort(-deg, kind="stable")
    bin_of = np.empty(n, dtype=np.int64)
    slot_of = np.empty(n, dtype=np.int64)
    counts = np.zeros(nbins, dtype=np.int64)
    loads = np.zeros(nbins, dtype=np.int64)
    heap = [(0, i) for i in range(nbins)]
    heapq.heapify(heap)
    for nd in order:
        while True:
            load, b = heapq.heappop(heap)
            if counts[b] < cap:
                break
        bin_of[nd] = b
        slot_of[nd] = counts[b]
        counts[b] += 1
        loads[b] += deg[nd]
        if counts[b] < cap:
            heapq.heappush(heap, (loads[b], b))
    return bin_of, slot_of, loads


def prep_inputs(nft, eft, W_path, b_path, W_attn1, attn2, src, dst, npc):
    """Host-side sharding/relayout. Returns (in_maps, unperm, tpb, has_bias)."""
    n_nodes = nft.shape[0]
    nb = npc // 128
    nbins = NCORES * nb

    nft = np.ascontiguousarray(nft, dtype=np.float32)
    src = np.asarray(src, dtype=np.int64)
    dst = np.asarray(dst, dtype=np.int64)
    deg = np.bincount(dst, minlength=n_nodes)

    # --- node renumbering: balance per-block edge counts ---
    bin_of, slot_of, loads = _binpack(deg, nbins, 128)
    tpb = max(1, int(np.ceil(loads.max() / 128.0)))
    # assign bins to cores (LPT on loads, 20 bins per core)
    order = np.argsort(-loads, kind="stable")
    core_of_bin = np.empty(nbins, dtype=np.int64)
    block_of_bin = np.empty(nbins, dtype=np.int64)
    cheap = [(0, c) for c in range(NCORES)]
    heapq.heapify(cheap)
    ccount = np.zeros(NCORES, dtype=np.int64)
    for g in order:
        while True:
            load, c = heapq.heappop(cheap)
            if ccount[c] < nb:
                break
        core_of_bin[g] = c
        block_of_bin[g] = ccount[c]
        ccount[c] += 1
        load += loads[g]
        if ccount[c] < nb:
            heapq.heappush(cheap, (load, c))

    ntiles = nb * tpb
    epad = ntiles * 128
    npad = nbins * 128

    nft16 = nft.astype(np.float16)

    # attention combination weights
    a2 = np.asarray(attn2, dtype=np.float32).reshape(H, DH)
    A2blk = np.zeros((F, H), dtype=np.float32)
    for h in range(H):
        A2blk[h * DH:(h + 1) * DH, h] = a2[h]
    Wp = np.ascontiguousarray(W_path, dtype=np.float32)
    W1, W2, W3 = Wp[0:F], Wp[F:2 * F], Wp[2 * F:3 * F]
    w2cat = np.concatenate([W2, W2 @ A2blk], axis=1).astype(np.float16)
    w1qa = np.concatenate(
        [W1, np.asarray(W_attn1, np.float32) + W1 @ A2blk], axis=1
    ).astype(np.float16)
    w3_np = W3.astype(np.float16)

    has_bias = bool(np.any(np.asarray(b_path) != 0))
    b = np.asarray(b_path, dtype=np.float32).reshape(F)
    bqa = b @ A2blk  # folded into rdst
    # r[dst] logit term (y3@A2): tiny per-node table, gathered on host
    r_node = (nft @ (W3 @ A2blk)).astype(np.float32)  # [N, H]

    # --- edge routing ---
    core_of_e = core_of_bin[bin_of[dst]]
    sortkey = (core_of_e * nb + block_of_bin[bin_of[dst]]) * 128 + slot_of[dst]
    perm = np.argsort(sortkey, kind="stable")
    s_key = sortkey[perm]
    s_src = src[perm]
    s_dst = dst[perm]
    s_eft = np.asarray(eft)[perm]

    in_maps = []
    blkkey = s_key // 128  # global (core*nb + block) of each sorted edge
    for c in range(NCORES):
        eftT_c = np.zeros((F, epad), dtype=np.float16)
        hsT_c = np.zeros((F, epad), dtype=np.float16)
        dstloc = np.zeros(epad, dtype=np.int64)
        rdst = np.full((epad, H), -10000.0, dtype=np.float32)

        for b_i in range(nb):
            gk = c * nb + b_i
            s = np.searchsorted(blkkey, gk)
            e = np.searchsorted(blkkey, gk + 1)
            cnt = e - s
            assert cnt <= tpb * 128, f"block overflow: {cnt} > {tpb * 128}"
            o = b_i * tpb * 128
            eftT_c[:, o:o + cnt] = s_eft[s:e].T.astype(np.float16)
            hsT_c[:, o:o + cnt] = nft16[s_src[s:e]].T
            dstloc[o:o + cnt] = slot_of[s_dst[s:e]]
            rdst[o:o + cnt] = r_node[s_dst[s:e]] + bqa

        dstloc_cols = dstloc.reshape(ntiles, 128).T.astype(np.float16).copy()
        rdst_cat = np.ascontiguousarray(
            rdst.reshape(ntiles, 128, H).transpose(1, 0, 2).reshape(128, ntiles * H)
        ).astype(np.float16)

        # per-core node features (new order)
        ids = np.arange(npc, dtype=np.int64)
        gbin = c * nb + ids // 128
        rows = np.zeros((npc, F), dtype=np.float32)
        mask = np.zeros(npc, dtype=bool)
        # nodes whose (bin) is assigned to this core at block ids//128
        for b_i in range(nb):
            # find global bin g with core_of_bin[g]==c and block_of_bin[g]==b_i
            g = np.where((core_of_bin == c) & (block_of_bin == b_i))[0][0]
            sel = bin_of == g
            nds = np.where(sel)[0]
            sl = slot_of[nds]
            rows[b_i * 128 + sl] = nft[nds]
            mask[b_i * 128 + sl] = deg[nds] > 0
        nftT_c = rows.T.astype(np.float16).copy()
        nftT_cm = (rows * mask[:, None]).T.astype(np.float16).copy()

        m = {
            "eftT": eftT_c,
            "hsT": hsT_c,
            "nftT_c": nftT_c,
            "nftT_cm": nftT_cm,
            "w2cat": w2cat,
            "w1qa": w1qa,
            "w3": w3_np,
            "dstloc": dstloc_cols,
            "rdst": rdst_cat,
        }
        if has_bias:
            m["brow"] = b.astype(np.float16).reshape(1, F)
        in_maps.append(m)

    unperm = (core_of_bin, block_of_bin, bin_of, slot_of)
    return in_maps, unperm, tpb, has_bias


_NC_CACHE = {}


def _get_nc(key, *args, **kw):
    if key not in _NC_CACHE:
        _NC_CACHE[key] = build_nc(*args, **kw)
    return _NC_CACHE[key]


def run(nft, eft, W_path, b_path, W_attn1, attn2, src, dst, trace=False,
        tmpdir=None, prec="f16"):
    n_nodes = nft.shape[0]
    npc = ((n_nodes + NCORES - 1) // NCORES + 127) // 128 * 128
    nb = npc // 128

    in_maps, unperm, tpb, has_bias = prep_inputs(
        np.asarray(nft), np.asarray(eft), np.asarray(W_path),
        np.asarray(b_path), np.asarray(W_attn1), np.asarray(attn2),
        np.asarray(src), np.asarray(dst), npc)
    core_of_bin, block_of_bin, bin_of, slot_of = unperm
    npad = NCORES * nb * 128

    nc = _get_nc((npc, tpb, npad, has_bias), npc, tpb, npad, has_bias)
    kw = {}
    if trace:
        kw = dict(trace=True, tmpdir=tmpdir)
    res = bass_utils.run_bass_kernel_spmd(nc, in_maps,
                                          core_ids=list(range(NCORES)), **kw)

    out = np.empty((n_nodes, F), dtype=np.float32)
    cols = block_of_bin[bin_of] * 128 + slot_of  # column in owning core's outT
    cores = core_of_bin[bin_of]
    for c in range(NCORES):
        sel = cores == c
        out[sel] = res.results[c]["outT"][:, cols[sel]].T
    return out, res


def kernel(**inputs):
    out, _ = run(**inputs)
    return out


# revision 38
# speedup vs baseline: 1.3357x; 1.0608x over previous
"""GAT message-passing kernel for 8 Trainium2 NeuronCores (Bass/Tile).

Strategy ("route edges by dst ownership", no collectives):
  - Host renumbers nodes (LPT bin-packing by degree) into 160 blocks of <=128
    nodes so every 128-node block has nearly equal in-edge count; each core
    owns 20 blocks, so segment-softmax and scatter-sum are fully core-local.
  - Reassociation: epaths = y1[src] + eft@W2 + y3[dst] + b with y1 = nft@W1.
    Since sum(att)=1 per (node, head), the y3[dst] part of the aggregated
    message is exactly +y3[dst], added once per node in phase 3.  Softmax is
    computed without max-subtraction; a fixed shift exp(a-7) keeps the
    unnormalized weights in fp16 range (softmax is shift-invariant).
  - No node-table phase and no device-side gather: the src node features are
    gathered per edge on the HOST (the sharding hint: "each device holds the
    gathered src/dst features") into a dense transposed tensor hsT that the
    kernel streams with full-rate 2KB DMA descriptors, and y1/qa are computed
    per edge on the PE:
      psEP[:, k, 0:136] = eftT_tile.T @ [W2 | W2A2] + hsT_tile.T @ [W1 | Wqa]
    where Wqa = W_attn1 + W1@A2. The r[dst] = y3@A2 logit term is tiny
    (8 floats per node) and is gathered per edge on the host (rdst); dummy
    padding edges get rdst = -1e4 which drives their weight u to exp(<-100)=0.
  - Per tile: one-hot P (dst-slot x edge) is built on-chip by a DVE is_equal
    against an iota row; P is the stationary of the scatter matmul that
    accumulates [agg | s] for the owning 128-node block.
  - Phase 3 (per node block): agg/s, transpose via PE, += nft@W3 (matmul
    accumulate, deg>0-masked) and += nft, relu, store transposed output.
"""

import sys
import heapq
import numpy as np
import ml_dtypes

for _p in ("/opt/trn_rl_repo",):
    if _p not in sys.path:
        sys.path.append(_p)

import concourse.bacc as bacc
import concourse.bass as bass
import concourse.mybir as mybir
from concourse.tile import TileContext
from concourse import bass_utils

F = 128
H = 8
DH = 16
F2 = F + H  # 136
NCORES = 8
EXP_SHIFT = 7.0  # exp(a - shift); softmax-invariant, keeps u in fp16 range
BATCH = 3        # tiles per PSUM epilogue batch ([128, 3, 136] f32 <= 2KB bank)


def build_nc(npc, tpb, npad, has_bias, debug=False):
    nb = npc // 128                  # node blocks per core
    ntiles = nb * tpb                # edge tiles per core
    epad = ntiles * 128              # padded edge count per core
    dt = mybir.dt
    AOP = mybir.AluOpType

    nc = bacc.Bacc("TRN2", target_bir_lowering=False, debug=False,
                   num_devices=NCORES, num_swdge_queues=4)

    # ---- inputs ----
    eftT = nc.dram_tensor("eftT", (F, epad), dt.float16, kind="ExternalInput")
    hsT_in = nc.dram_tensor("hsT", (F, epad), dt.float16, kind="ExternalInput")
    nftT_c = nc.dram_tensor("nftT_c", (F, npc), dt.float16, kind="ExternalInput")
    nftT_cm = nc.dram_tensor("nftT_cm", (F, npc), dt.float16, kind="ExternalInput")
    w2cat_in = nc.dram_tensor("w2cat", (F, F2), dt.float16, kind="ExternalInput")
    w1qa_in = nc.dram_tensor("w1qa", (F, F2), dt.float16, kind="ExternalInput")
    w3_in = nc.dram_tensor("w3", (F, F), dt.float16, kind="ExternalInput")
    dstloc_in = nc.dram_tensor("dstloc", (128, ntiles), dt.bfloat16, kind="ExternalInput")
    rdst_in = nc.dram_tensor("rdst", (128, ntiles * H), dt.float16, kind="ExternalInput")
    if has_bias:
        brow_in = nc.dram_tensor("brow", (1, F), dt.float16, kind="ExternalInput")

    outT = nc.dram_tensor("outT", (F, npc), dt.float32, kind="ExternalOutput")

    with TileContext(nc) as tc:
        with tc.tile_pool(name="const", bufs=1) as cpool, \
             tc.tile_pool(name="work", bufs=3) as pool, \
             tc.tile_pool(name="io", bufs=4) as iop, \
             tc.tile_pool(name="psEP", bufs=3, space="PSUM") as psEP, \
             tc.tile_pool(name="psB", bufs=2, space="PSUM") as psB, \
             tc.tile_pool(name="psC", bufs=2, space="PSUM") as psC:

            # ---------- constants ----------
            iota_row = cpool.tile([128, 128], dt.float32)
            nc.gpsimd.iota(iota_row, pattern=[[1, 128]], channel_multiplier=0,
                           allow_small_or_imprecise_dtypes=True)
            iota_col = cpool.tile([128, 1], dt.float32)
            nc.gpsimd.iota(iota_col, pattern=[[1, 1]], channel_multiplier=1,
                           allow_small_or_imprecise_dtypes=True)
            ident = cpool.tile([128, 128], dt.float32)
            nc.vector.tensor_scalar(out=ident, in0=iota_row[:, :],
                                    scalar1=iota_col[:, :], scalar2=None,
                                    op0=AOP.is_equal)
            nshift = cpool.tile([128, 1], dt.float32)
            nc.vector.memset(nshift, -EXP_SHIFT)
            # 0..127 repeated per chunk tile, unit-stride (for one-hot build)
            iota_big = cpool.tile([128, 32 * 128], dt.bfloat16)
            nc.gpsimd.iota(iota_big, pattern=[[0, 32], [1, 128]],
                           channel_multiplier=0,
                           allow_small_or_imprecise_dtypes=True)

            w2cat_s = cpool.tile([F, F2], dt.float16)
            nc.sync.dma_start(out=w2cat_s, in_=w2cat_in[:, :])
            w1qa_s = cpool.tile([F, F2], dt.float16)
            nc.sync.dma_start(out=w1qa_s, in_=w1qa_in[:, :])
            w3_s = cpool.tile([F, F], dt.float16)
            nc.sync.dma_start(out=w3_s, in_=w3_in[:, :])

            if has_bias:
                brow_s = cpool.tile([1, F], dt.float16)
                nc.sync.dma_start(out=brow_s, in_=brow_in[:, :])
                ones_row = cpool.tile([1, 128], dt.float16)
                nc.vector.memset(ones_row, 1.0)

            dstloc_s = cpool.tile([128, ntiles], dt.bfloat16)
            nc.sync.dma_start(out=dstloc_s, in_=dstloc_in[:, :])
            rdst_s = cpool.tile([128, ntiles * H], dt.float16)
            nc.sync.dma_start(out=rdst_s, in_=rdst_in[:, :])
            nftc_s = cpool.tile([128, npc], dt.float16)
            nc.sync.dma_start(out=nftc_s, in_=nftT_c[:, :])
            nftcm_s = cpool.tile([128, npc], dt.float16)
            nc.sync.dma_start(out=nftcm_s, in_=nftT_cm[:, :])

            # ---------- main loop over edge tiles ----------
            psb_cur = None
            eft_ld = hsT = None
            psa = None
            CH = 32  # tiles per DMA chunk (8KB per partition per stream)
            pch_hist = {}
            for t in range(ntiles):
                b, j = divmod(t, tpb)
                tB = t % BATCH
                tc16 = t % CH
                if tc16 == 0:
                    w = min(CH * 128, (ntiles - t) * 128)
                    nch16 = w // 128
                    eft_ld = iop.tile([128, CH * 128], dt.float16, tag="eft")
                    nc.sync.dma_start(out=eft_ld[:, 0:w],
                                      in_=eftT[:, t * 128:t * 128 + w])
                    hsT = iop.tile([128, CH * 128], dt.float16, tag="hsT")
                    nc.sync.dma_start(out=hsT[:, 0:w],
                                      in_=hsT_in[:, t * 128:t * 128 + w])
                    # one-hot P for the whole chunk in one DVE op (bf16: 2x uop)
                    pchunk = iop.tile([128, CH, 128], dt.bfloat16, tag="pchunk")
                    nc.vector.tensor_tensor(
                        out=pchunk[:, 0:nch16, :],
                        in0=dstloc_s[:, t:t + nch16, None]
                            .broadcast_to((128, nch16, 128)),
                        in1=iota_big[:, 0:nch16 * 128].rearrange(
                            "p (k c) -> p k c", c=128),
                        op=AOP.is_equal)
                    pch_hist[t // CH] = pchunk
                    pch_hist.pop(t // CH - 2, None)
                if tB == 0:
                    psa = psEP.tile([128, BATCH, F2], dt.float32, tag="ep")

                et = eft_ld[:, tc16 * 128:(tc16 + 1) * 128]
                hst = hsT[:, tc16 * 128:(tc16 + 1) * 128]
                nc.tensor.matmul(psa[:, tB, :], lhsT=et, rhs=w2cat_s,
                                 start=True, stop=False, skip_group_check=True)
                nc.tensor.matmul(psa[:, tB, :], lhsT=hst, rhs=w1qa_s,
                                 start=False, stop=not has_bias,
                                 skip_group_check=True)
                if has_bias:
                    nc.tensor.matmul(psa[:, tB, 0:F], lhsT=ones_row, rhs=brow_s,
                                     start=False, stop=True,
                                     skip_group_check=True)

                if tB != (min(BATCH, ntiles - t + tB) - 1):
                    continue
                # ---- batch epilogue: n4 tiles of logits and messages ----
                n4 = tB + 1
                tb = t - tB
                z4 = pool.tile([128, BATCH, H], dt.float32, tag="z4")
                nc.vector.tensor_tensor(
                    out=z4[:, 0:n4, :],
                    in0=psa[:, 0:n4, F:F2],
                    in1=rdst_s[:, tb * H:(tb + n4) * H].rearrange(
                        "p (k h) -> p k h", h=H),
                    op=AOP.add)
                a4 = pool.tile([128, BATCH, H], dt.float32, tag="a4")
                nc.vector.scalar_tensor_tensor(
                    out=a4[:, 0:n4, :], in0=z4[:, 0:n4, :], scalar=0.01,
                    in1=z4[:, 0:n4, :], op0=AOP.mult, op1=AOP.max)
                msgu4 = pool.tile([128, BATCH, F2], dt.float16, tag="msgu4")
                nc.scalar.activation(
                    msgu4[:, 0:n4, F:F2], a4[:, 0:n4, :],
                    mybir.ActivationFunctionType.Exp,
                    bias=nshift[:, :])
                nc.vector.tensor_tensor(
                    out=msgu4[:, 0:n4, 0:F].rearrange("p k (h d) -> p k h d", h=H),
                    in0=psa[:, 0:n4, 0:F].rearrange("p k (h d) -> p k h d", h=H),
                    in1=msgu4[:, 0:n4, F:F2][:, :, :, None]
                        .broadcast_to((128, n4, H, DH)),
                    op=AOP.mult)

                # scatter each tile of the batch into its block accumulator
                for k in range(n4):
                    tg = tb + k
                    bb, jj = divmod(tg, tpb)
                    pk = pch_hist[tg // CH][:, tg % CH, :]
                    if jj == 0:
                        psb_cur = psB.tile([128, F2], dt.float32, tag="aggB")
                    nc.tensor.matmul(psb_cur, lhsT=pk, rhs=msgu4[:, k, :],
                                     start=(jj == 0), stop=(jj == tpb - 1),
                                     skip_group_check=True)
                    if jj != tpb - 1:
                        continue
                    # ---------- phase 3 for block bb ----------
                    ss = pool.tile([128, H], dt.float32, tag="ss")
                    nc.vector.tensor_scalar(out=ss, in0=psb_cur[:, F:F2],
                                            scalar1=1e-30, scalar2=None,
                                            op0=AOP.max)
                    inv = pool.tile([128, H], dt.float32, tag="inv")
                    nc.vector.reciprocal(inv, ss)
                    mn = pool.tile([128, F], dt.float32, tag="mn")
                    nc.vector.tensor_tensor(
                        out=mn[:, :].rearrange("p (h d) -> p h d", h=H),
                        in0=psb_cur[:, 0:F].rearrange("p (h d) -> p h d", h=H),
                        in1=inv[:, :, None].broadcast_to((128, H, DH)),
                        op=AOP.mult)
                    psc = psC.tile([128, 128], dt.float32, tag="outC")
                    nc.tensor.matmul(psc, lhsT=w3_s,
                                     rhs=nftcm_s[:, bb * 128:(bb + 1) * 128],
                                     start=True, stop=False)
                    nc.tensor.matmul(psc, lhsT=mn, rhs=ident,
                                     is_transpose=True,
                                     start=False, stop=True)
                    oc = pool.tile([128, 128], dt.float32, tag="oc")
                    nc.vector.tensor_tensor(
                        out=oc, in0=psc,
                        in1=nftc_s[:, bb * 128:(bb + 1) * 128], op=AOP.add)
                    oc2 = pool.tile([128, 128], dt.float32, tag="oc2")
                    nc.scalar.activation(oc2, oc,
                                         mybir.ActivationFunctionType.Relu)
                    nc.sync.dma_start(out=outT[:, bb * 128:(bb + 1) * 128],
                                      in_=oc2)

    nc.compile()
    return nc


def _binpack(deg, nbins, cap):
    """LPT bin-packing of nodes into nbins bins of <= cap nodes, minimizing
    the max per-bin degree sum. Returns (bin_of_node, slot_of_node, loads)."""
    n = len(deg)
    order = np.argsort(-deg, kind="stable")
    bin_of = np.empty(n, dtype=np.int64)
    slot_of = np.empty(n, dtype=np.int64)
    counts = np.zeros(nbins, dtype=np.int64)
    loads = np.zeros(nbins, dtype=np.int64)
    heap = [(0, i) for i in range(nbins)]
    heapq.heapify(heap)
    for nd in order:
        while True:
            load, b = heapq.heappop(heap)
            if counts[b] < cap:
                break
        bin_of[nd] = b
        slot_of[nd] = counts[b]
        counts[b] += 1
        loads[b] += deg[nd]
        if counts[b] < cap:
            heapq.heappush(heap, (loads[b], b))
    return bin_of, slot_of, loads


def prep_inputs(nft, eft, W_path, b_path, W_attn1, attn2, src, dst, npc):
    """Host-side sharding/relayout. Returns (in_maps, unperm, tpb, has_bias)."""
    n_nodes = nft.shape[0]
    nb = npc // 128
    nbins = NCORES * nb

    nft = np.ascontiguousarray(nft, dtype=np.float32)
    src = np.asarray(src, dtype=np.int64)
    dst = np.asarray(dst, dtype=np.int64)
    deg = np.bincount(dst, minlength=n_nodes)

    # --- node renumbering: balance per-block edge counts ---
    bin_of, slot_of, loads = _binpack(deg, nbins, 128)
    tpb = max(1, int(np.ceil(loads.max() / 128.0)))
    # assign bins to cores (LPT on loads, 20 bins per core)
    order = np.argsort(-loads, kind="stable")
    core_of_bin = np.empty(nbins, dtype=np.int64)
    block_of_bin = np.empty(nbins, dtype=np.int64)
    cheap = [(0, c) for c in range(NCORES)]
    heapq.heapify(cheap)
    ccount = np.zeros(NCORES, dtype=np.int64)
    for g in order:
        while True:
            load, c = heapq.heappop(cheap)
            if ccount[c] < nb:
                break
        core_of_bin[g] = c
        block_of_bin[g] = ccount[c]
        ccount[c] += 1
        load += loads[g]
        if ccount[c] < nb:
            heapq.heappush(cheap, (load, c))

    ntiles = nb * tpb
    epad = ntiles * 128
    npad = nbins * 128

    nft16 = nft.astype(np.float16)

    # attention combination weights
    a2 = np.asarray(attn2, dtype=np.float32).reshape(H, DH)
    A2blk = np.zeros((F, H), dtype=np.float32)
    for h in range(H):
        A2blk[h * DH:(h + 1) * DH, h] = a2[h]
    Wp = np.ascontiguousarray(W_path, dtype=np.float32)
    W1, W2, W3 = Wp[0:F], Wp[F:2 * F], Wp[2 * F:3 * F]
    w2cat = np.concatenate([W2, W2 @ A2blk], axis=1).astype(np.float16)
    w1qa = np.concatenate(
        [W1, np.asarray(W_attn1, np.float32) + W1 @ A2blk], axis=1
    ).astype(np.float16)
    w3_np = W3.astype(np.float16)

    has_bias = bool(np.any(np.asarray(b_path) != 0))
    b = np.asarray(b_path, dtype=np.float32).reshape(F)
    bqa = b @ A2blk  # folded into rdst
    # r[dst] logit term (y3@A2): tiny per-node table, gathered on host
    r_node = (nft @ (W3 @ A2blk)).astype(np.float32)  # [N, H]

    # --- edge routing ---
    core_of_e = core_of_bin[bin_of[dst]]
    sortkey = (core_of_e * nb + block_of_bin[bin_of[dst]]) * 128 + slot_of[dst]
    perm = np.argsort(sortkey, kind="stable")
    s_key = sortkey[perm]
    s_src = src[perm]
    s_dst = dst[perm]
    s_eft = np.asarray(eft)[perm]

    in_maps = []
    blkkey = s_key // 128  # global (core*nb + block) of each sorted edge
    for c in range(NCORES):
        eftT_c = np.zeros((F, epad), dtype=np.float16)
        hsT_c = np.zeros((F, epad), dtype=np.float16)
        dstloc = np.zeros(epad, dtype=np.int64)
        rdst = np.full((epad, H), -10000.0, dtype=np.float32)

        for b_i in range(nb):
            gk = c * nb + b_i
            s = np.searchsorted(blkkey, gk)
            e = np.searchsorted(blkkey, gk + 1)
            cnt = e - s
            assert cnt <= tpb * 128, f"block overflow: {cnt} > {tpb * 128}"
            o = b_i * tpb * 128
            eftT_c[:, o:o + cnt] = s_eft[s:e].T.astype(np.float16)
            hsT_c[:, o:o + cnt] = nft16[s_src[s:e]].T
            dstloc[o:o + cnt] = slot_of[s_dst[s:e]]
            rdst[o:o + cnt] = r_node[s_dst[s:e]] + bqa

        dstloc_cols = dstloc.reshape(ntiles, 128).T.astype(
            ml_dtypes.bfloat16).copy()
        rdst_cat = np.ascontiguousarray(
            rdst.reshape(ntiles, 128, H).transpose(1, 0, 2).reshape(128, ntiles * H)
        ).astype(np.float16)

        # per-core node features (new order)
        ids = np.arange(npc, dtype=np.int64)
        gbin = c * nb + ids // 128
        rows = np.zeros((npc, F), dtype=np.float32)
        mask = np.zeros(npc, dtype=bool)
        # nodes whose (bin) is assigned to this core at block ids//128
        for b_i in range(nb):
            # find global bin g with core_of_bin[g]==c and block_of_bin[g]==b_i
            g = np.where((core_of_bin == c) & (block_of_bin == b_i))[0][0]
            sel = bin_of == g
            nds = np.where(sel)[0]
            sl = slot_of[nds]
            rows[b_i * 128 + sl] = nft[nds]
            mask[b_i * 128 + sl] = deg[nds] > 0
        nftT_c = rows.T.astype(np.float16).copy()
        nftT_cm = (rows * mask[:, None]).T.astype(np.float16).copy()

        m = {
            "eftT": eftT_c,
            "hsT": hsT_c,
            "nftT_c": nftT_c,
            "nftT_cm": nftT_cm,
            "w2cat": w2cat,
            "w1qa": w1qa,
            "w3": w3_np,
            "dstloc": dstloc_cols,
            "rdst": rdst_cat,
        }
        if has_bias:
            m["brow"] = b.astype(np.float16).reshape(1, F)
        in_maps.append(m)

    unperm = (core_of_bin, block_of_bin, bin_of, slot_of)
    return in_maps, unperm, tpb, has_bias


_NC_CACHE = {}


def _get_nc(key, *args, **kw):
    if key not in _NC_CACHE:
        _NC_CACHE[key] = build_nc(*args, **kw)
    return _NC_CACHE[key]


def run(nft, eft, W_path, b_path, W_attn1, attn2, src, dst, trace=False,
        tmpdir=None, prec="f16"):
    n_nodes = nft.shape[0]
    npc = ((n_nodes + NCORES - 1) // NCORES + 127) // 128 * 128
    nb = npc // 128

    in_maps, unperm, tpb, has_bias = prep_inputs(
        np.asarray(nft), np.asarray(eft), np.asarray(W_path),
        np.asarray(b_path), np.asarray(W_attn1), np.asarray(attn2),
        np.asarray(src), np.asarray(dst), npc)
    core_of_bin, block_of_bin, bin_of, slot_of = unperm
    npad = NCORES * nb * 128

    nc = _get_nc((npc, tpb, npad, has_bias), npc, tpb, npad, has_bias)
    kw = {}
    if trace:
        kw = dict(trace=True, tmpdir=tmpdir)
    res = bass_utils.run_bass_kernel_spmd(nc, in_maps,
                                          core_ids=list(range(NCORES)), **kw)

    out = np.empty((n_nodes, F), dtype=np.float32)
    cols = block_of_bin[bin_of] * 128 + slot_of  # column in owning core's outT
    cores = core_of_bin[bin_of]
    for c in range(NCORES):
        sel = cores == c
        out[sel] = res.results[c]["outT"][:, cols[sel]].T
    return out, res


def kernel(**inputs):
    out, _ = run(**inputs)
    return out


# revision 46
# speedup vs baseline: 1.7327x; 1.2972x over previous
"""GAT message-passing kernel for 8 Trainium2 NeuronCores (Bass/Tile).

Strategy ("route edges by dst ownership", no collectives):
  - Host renumbers nodes (LPT bin-packing by degree) into 160 blocks of <=128
    nodes so every 128-node block has nearly equal in-edge count; each core
    owns 20 blocks, so segment-softmax and scatter-sum are fully core-local.
  - Reassociation: epaths = y1[src] + eft@W2 + y3[dst] + b with y1 = nft@W1.
    Since sum(att)=1 per (node, head), the y3[dst] part of the aggregated
    message is exactly +y3[dst], added once per node in phase 3.  Softmax is
    computed without max-subtraction; a fixed shift exp(a-7) keeps the
    unnormalized weights in fp16 range (softmax is shift-invariant).
  - No node-table phase and no device-side gather: the src node features are
    gathered per edge on the HOST (the sharding hint: "each device holds the
    gathered src/dst features") into a dense transposed tensor hsT that the
    kernel streams with full-rate 2KB DMA descriptors, and y1/qa are computed
    per edge on the PE:
      psEP[:, k, 0:136] = eftT_tile.T @ [W2 | W2A2] + hsT_tile.T @ [W1 | Wqa]
    where Wqa = W_attn1 + W1@A2. The r[dst] = y3@A2 logit term is tiny
    (8 floats per node) and is gathered per edge on the host (rdst); dummy
    padding edges get rdst = -1e4 which drives their weight u to exp(<-100)=0.
  - Per tile: one-hot P (dst-slot x edge) is built on-chip by a DVE is_equal
    against an iota row; P is the stationary of the scatter matmul that
    accumulates [agg | s] for the owning 128-node block.
  - Phase 3 (per node block): agg/s, transpose via PE, += nft@W3 (matmul
    accumulate, deg>0-masked) and += nft, relu, store transposed output.
"""

import sys
import heapq
import numpy as np
import ml_dtypes

for _p in ("/opt/trn_rl_repo",):
    if _p not in sys.path:
        sys.path.append(_p)

import concourse.bacc as bacc
import concourse.bass as bass
import concourse.mybir as mybir
from concourse.tile import TileContext
from concourse import bass_utils

F = 128
H = 8
DH = 16
F2 = F + H  # 136
NCORES = 8
EXP_SHIFT = 7.0  # exp(a - shift); softmax-invariant, keeps u in fp16 range
BATCH = 3        # tiles per PSUM epilogue batch ([128, 3, 136] f32 <= 2KB bank)


def build_nc(npc, tpb, npad, has_bias, debug=False):
    nb = npc // 128                  # node blocks per core
    ntiles = nb * tpb                # edge tiles per core
    epad = ntiles * 128              # padded edge count per core
    dt = mybir.dt
    AOP = mybir.AluOpType

    nc = bacc.Bacc("TRN2", target_bir_lowering=False, debug=False,
                   num_devices=NCORES, num_swdge_queues=4)

    # ---- inputs ----
    eftT = nc.dram_tensor("eftT", (F, epad), dt.float16, kind="ExternalInput")
    hsT_in = nc.dram_tensor("hsT", (F, epad), dt.float16, kind="ExternalInput")
    nftT_c = nc.dram_tensor("nftT_c", (F, npc), dt.float16, kind="ExternalInput")
    nftT_cm = nc.dram_tensor("nftT_cm", (F, npc), dt.float16, kind="ExternalInput")
    w2cat_in = nc.dram_tensor("w2cat", (F, F2), dt.float16, kind="ExternalInput")
    w1qa_in = nc.dram_tensor("w1qa", (F, F2), dt.float16, kind="ExternalInput")
    w3_in = nc.dram_tensor("w3", (F, F), dt.float16, kind="ExternalInput")
    Pcat = nc.dram_tensor("Pcat", (128, epad), dt.float8e4, kind="ExternalInput")
    rdst_in = nc.dram_tensor("rdst", (128, ntiles * H), dt.float16, kind="ExternalInput")
    if has_bias:
        brow_in = nc.dram_tensor("brow", (1, F), dt.float16, kind="ExternalInput")

    outT = nc.dram_tensor("outT", (F, npc), dt.float32, kind="ExternalOutput")

    with TileContext(nc) as tc:
        with tc.tile_pool(name="const", bufs=1) as cpool, \
             tc.tile_pool(name="work", bufs=3) as pool, \
             tc.tile_pool(name="io", bufs=4) as iop, \
             tc.tile_pool(name="psEP", bufs=3, space="PSUM") as psEP, \
             tc.tile_pool(name="psB", bufs=2, space="PSUM") as psB, \
             tc.tile_pool(name="psC", bufs=2, space="PSUM") as psC:

            # ---------- constants ----------
            iota_row = cpool.tile([128, 128], dt.float32)
            nc.gpsimd.iota(iota_row, pattern=[[1, 128]], channel_multiplier=0,
                           allow_small_or_imprecise_dtypes=True)
            iota_col = cpool.tile([128, 1], dt.float32)
            nc.gpsimd.iota(iota_col, pattern=[[1, 1]], channel_multiplier=1,
                           allow_small_or_imprecise_dtypes=True)
            ident = cpool.tile([128, 128], dt.float32)
            nc.vector.tensor_scalar(out=ident, in0=iota_row[:, :],
                                    scalar1=iota_col[:, :], scalar2=None,
                                    op0=AOP.is_equal)
            nshift = cpool.tile([128, 1], dt.float32)
            nc.vector.memset(nshift, -EXP_SHIFT)
            ident16 = cpool.tile([128, 128], dt.float16)
            nc.vector.tensor_copy(out=ident16, in_=ident)

            w2cat_s = cpool.tile([F, F2], dt.float16)
            nc.sync.dma_start(out=w2cat_s, in_=w2cat_in[:, :])
            w1qa_s = cpool.tile([F, F2], dt.float16)
            nc.sync.dma_start(out=w1qa_s, in_=w1qa_in[:, :])
            w3_s = cpool.tile([F, F], dt.float16)
            nc.sync.dma_start(out=w3_s, in_=w3_in[:, :])

            if has_bias:
                brow_s = cpool.tile([1, F], dt.float16)
                nc.sync.dma_start(out=brow_s, in_=brow_in[:, :])
                ones_row = cpool.tile([1, 128], dt.float16)
                nc.vector.memset(ones_row, 1.0)

            rdst_s = cpool.tile([128, ntiles * H], dt.float16)
            nc.sync.dma_start(out=rdst_s, in_=rdst_in[:, :])
            nftc_s = cpool.tile([128, npc], dt.float16)
            nc.sync.dma_start(out=nftc_s, in_=nftT_c[:, :])
            nftcm_s = cpool.tile([128, npc], dt.float16)
            nc.sync.dma_start(out=nftcm_s, in_=nftT_cm[:, :])

            # ---------- main loop over edge tiles ----------
            psb_cur = None
            eft_ld = hsT = None
            psa = None
            CH = 32  # tiles per DMA chunk (8KB per partition per stream)
            pch_hist = {}
            for t in range(ntiles):
                b, j = divmod(t, tpb)
                tB = t % BATCH
                tc16 = t % CH
                if tc16 == 0:
                    w = min(CH * 128, (ntiles - t) * 128)
                    nch16 = w // 128
                    eft_ld = iop.tile([128, CH * 128], dt.float16, tag="eft")
                    nc.sync.dma_start(out=eft_ld[:, 0:w],
                                      in_=eftT[:, t * 128:t * 128 + w])
                    hsT = iop.tile([128, CH * 128], dt.float16, tag="hsT")
                    nc.sync.dma_start(out=hsT[:, 0:w],
                                      in_=hsT_in[:, t * 128:t * 128 + w])
                    # host-built one-hot P (fp8), streamed like eft/hs
                    pchunk = iop.tile([128, CH * 128], dt.float8e4, tag="pchunk")
                    nc.sync.dma_start(out=pchunk[:, 0:w],
                                      in_=Pcat[:, t * 128:t * 128 + w])
                    pch_hist[t // CH] = pchunk
                    pch_hist.pop(t // CH - 2, None)
                if tB == 0:
                    psa = psEP.tile([128, BATCH, F2], dt.float32, tag="ep")

                et = eft_ld[:, tc16 * 128:(tc16 + 1) * 128]
                hst = hsT[:, tc16 * 128:(tc16 + 1) * 128]
                nc.tensor.matmul(psa[:, tB, :], lhsT=et, rhs=w2cat_s,
                                 start=True, stop=False, skip_group_check=True)
                nc.tensor.matmul(psa[:, tB, :], lhsT=hst, rhs=w1qa_s,
                                 start=False, stop=not has_bias,
                                 skip_group_check=True)
                if has_bias:
                    nc.tensor.matmul(psa[:, tB, 0:F], lhsT=ones_row, rhs=brow_s,
                                     start=False, stop=True,
                                     skip_group_check=True)

                if tB != (min(BATCH, ntiles - t + tB) - 1):
                    continue
                # ---- batch epilogue: n4 tiles of logits and messages ----
                n4 = tB + 1
                tb = t - tB
                z4 = pool.tile([128, BATCH, H], dt.float32, tag="z4")
                nc.vector.tensor_tensor(
                    out=z4[:, 0:n4, :],
                    in0=psa[:, 0:n4, F:F2],
                    in1=rdst_s[:, tb * H:(tb + n4) * H].rearrange(
                        "p (k h) -> p k h", h=H),
                    op=AOP.add)
                a4 = pool.tile([128, BATCH, H], dt.float32, tag="a4")
                nc.vector.scalar_tensor_tensor(
                    out=a4[:, 0:n4, :], in0=z4[:, 0:n4, :], scalar=0.01,
                    in1=z4[:, 0:n4, :], op0=AOP.mult, op1=AOP.max)
                msgu4 = pool.tile([128, BATCH, F2], dt.float16, tag="msgu4")
                nc.scalar.activation(
                    msgu4[:, 0:n4, F:F2], a4[:, 0:n4, :],
                    mybir.ActivationFunctionType.Exp,
                    bias=nshift[:, :])
                nc.vector.tensor_tensor(
                    out=msgu4[:, 0:n4, 0:F].rearrange("p k (h d) -> p k h d", h=H),
                    in0=psa[:, 0:n4, 0:F].rearrange("p k (h d) -> p k h d", h=H),
                    in1=msgu4[:, 0:n4, F:F2][:, :, :, None]
                        .broadcast_to((128, n4, H, DH)),
                    op=AOP.mult)

                # scatter each tile of the batch into its block accumulator
                for k in range(n4):
                    tg = tb + k
                    bb, jj = divmod(tg, tpb)
                    pk = pch_hist[tg // CH][:, (tg % CH) * 128:
                                            (tg % CH + 1) * 128]
                    if jj == 0:
                        psb_cur = psB.tile([128, F2], dt.float32, tag="aggB")
                    nc.tensor.matmul(psb_cur, lhsT=pk, rhs=msgu4[:, k, :],
                                     start=(jj == 0), stop=(jj == tpb - 1),
                                     skip_group_check=True)
                    if jj != tpb - 1:
                        continue
                    # ---------- phase 3 for block bb ----------
                    ss = pool.tile([128, H], dt.float32, tag="ss")
                    nc.vector.tensor_scalar(out=ss, in0=psb_cur[:, F:F2],
                                            scalar1=1e-30, scalar2=None,
                                            op0=AOP.max)
                    inv = pool.tile([128, H], dt.float32, tag="inv")
                    nc.vector.reciprocal(inv, ss)
                    mn = pool.tile([128, F], dt.float32, tag="mn")
                    nc.vector.tensor_tensor(
                        out=mn[:, :].rearrange("p (h d) -> p h d", h=H),
                        in0=psb_cur[:, 0:F].rearrange("p (h d) -> p h d", h=H),
                        in1=inv[:, :, None].broadcast_to((128, H, DH)),
                        op=AOP.mult)
                    psc = psC.tile([128, 128], dt.float32, tag="outC")
                    nc.tensor.matmul(psc, lhsT=w3_s,
                                     rhs=nftcm_s[:, bb * 128:(bb + 1) * 128],
                                     start=True, stop=False)
                    nc.tensor.matmul(psc, lhsT=mn, rhs=ident,
                                     is_transpose=True,
                                     start=False, stop=False)
                    nc.tensor.matmul(psc, lhsT=ident16,
                                     rhs=nftc_s[:, bb * 128:(bb + 1) * 128],
                                     start=False, stop=True)
                    oc2 = pool.tile([128, 128], dt.float32, tag="oc2")
                    nc.scalar.activation(oc2, psc,
                                         mybir.ActivationFunctionType.Relu)
                    nc.sync.dma_start(out=outT[:, bb * 128:(bb + 1) * 128],
                                      in_=oc2)

    nc.compile()
    return nc


def _binpack(deg, nbins, cap):
    """LPT bin-packing of nodes into nbins bins of <= cap nodes, minimizing
    the max per-bin degree sum. Returns (bin_of_node, slot_of_node, loads)."""
    n = len(deg)
    order = np.argsort(-deg, kind="stable")
    bin_of = np.empty(n, dtype=np.int64)
    slot_of = np.empty(n, dtype=np.int64)
    counts = np.zeros(nbins, dtype=np.int64)
    loads = np.zeros(nbins, dtype=np.int64)
    heap = [(0, i) for i in range(nbins)]
    heapq.heapify(heap)
    for nd in order:
        while True:
            load, b = heapq.heappop(heap)
            if counts[b] < cap:
                break
        bin_of[nd] = b
        slot_of[nd] = counts[b]
        counts[b] += 1
        loads[b] += deg[nd]
        if counts[b] < cap:
            heapq.heappush(heap, (loads[b], b))
    return bin_of, slot_of, loads


def prep_inputs(nft, eft, W_path, b_path, W_attn1, attn2, src, dst, npc):
    """Host-side sharding/relayout. Returns (in_maps, unperm, tpb, has_bias)."""
    n_nodes = nft.shape[0]
    nb = npc // 128
    nbins = NCORES * nb

    nft = np.ascontiguousarray(nft, dtype=np.float32)
    src = np.asarray(src, dtype=np.int64)
    dst = np.asarray(dst, dtype=np.int64)
    deg = np.bincount(dst, minlength=n_nodes)

    # --- node renumbering: balance per-block edge counts ---
    bin_of, slot_of, loads = _binpack(deg, nbins, 128)
    tpb = max(1, int(np.ceil(loads.max() / 128.0)))
    # assign bins to cores (LPT on loads, 20 bins per core)
    order = np.argsort(-loads, kind="stable")
    core_of_bin = np.empty(nbins, dtype=np.int64)
    block_of_bin = np.empty(nbins, dtype=np.int64)
    cheap = [(0, c) for c in range(NCORES)]
    heapq.heapify(cheap)
    ccount = np.zeros(NCORES, dtype=np.int64)
    for g in order:
        while True:
            load, c = heapq.heappop(cheap)
            if ccount[c] < nb:
                break
        core_of_bin[g] = c
        block_of_bin[g] = ccount[c]
        ccount[c] += 1
        load += loads[g]
        if ccount[c] < nb:
            heapq.heappush(cheap, (load, c))

    ntiles = nb * tpb
    epad = ntiles * 128
    npad = nbins * 128

    nft16 = nft.astype(np.float16)

    # attention combination weights
    a2 = np.asarray(attn2, dtype=np.float32).reshape(H, DH)
    A2blk = np.zeros((F, H), dtype=np.float32)
    for h in range(H):
        A2blk[h * DH:(h + 1) * DH, h] = a2[h]
    Wp = np.ascontiguousarray(W_path, dtype=np.float32)
    W1, W2, W3 = Wp[0:F], Wp[F:2 * F], Wp[2 * F:3 * F]
    w2cat = np.concatenate([W2, W2 @ A2blk], axis=1).astype(np.float16)
    w1qa = np.concatenate(
        [W1, np.asarray(W_attn1, np.float32) + W1 @ A2blk], axis=1
    ).astype(np.float16)
    w3_np = W3.astype(np.float16)

    has_bias = bool(np.any(np.asarray(b_path) != 0))
    b = np.asarray(b_path, dtype=np.float32).reshape(F)
    bqa = b @ A2blk  # folded into rdst
    # r[dst] logit term (y3@A2): tiny per-node table, gathered on host
    r_node = (nft @ (W3 @ A2blk)).astype(np.float32)  # [N, H]

    # --- edge routing ---
    core_of_e = core_of_bin[bin_of[dst]]
    sortkey = (core_of_e * nb + block_of_bin[bin_of[dst]]) * 128 + slot_of[dst]
    perm = np.argsort(sortkey, kind="stable")
    s_key = sortkey[perm]
    s_src = src[perm]
    s_dst = dst[perm]
    s_eft = np.asarray(eft)[perm]

    in_maps = []
    blkkey = s_key // 128  # global (core*nb + block) of each sorted edge
    for c in range(NCORES):
        eftT_c = np.zeros((F, epad), dtype=np.float16)
        hsT_c = np.zeros((F, epad), dtype=np.float16)
        dstloc = np.zeros(epad, dtype=np.int64)
        rdst = np.full((epad, H), -10000.0, dtype=np.float32)

        for b_i in range(nb):
            gk = c * nb + b_i
            s = np.searchsorted(blkkey, gk)
            e = np.searchsorted(blkkey, gk + 1)
            cnt = e - s
            assert cnt <= tpb * 128, f"block overflow: {cnt} > {tpb * 128}"
            o = b_i * tpb * 128
            eftT_c[:, o:o + cnt] = s_eft[s:e].T.astype(np.float16)
            hsT_c[:, o:o + cnt] = nft16[s_src[s:e]].T
            dstloc[o:o + cnt] = slot_of[s_dst[s:e]]
            rdst[o:o + cnt] = r_node[s_dst[s:e]] + bqa

        ee = np.arange(epad)
        Pcat_c = np.zeros((128, epad), dtype=mybir.dt.np(mybir.dt.float8e4))
        Pcat_c[ee % 128, (ee // 128) * 128 + dstloc] = 1.0
        rdst_cat = np.ascontiguousarray(
            rdst.reshape(ntiles, 128, H).transpose(1, 0, 2).reshape(128, ntiles * H)
        ).astype(np.float16)

        # per-core node features (new order)
        ids = np.arange(npc, dtype=np.int64)
        gbin = c * nb + ids // 128
        rows = np.zeros((npc, F), dtype=np.float32)
        mask = np.zeros(npc, dtype=bool)
        # nodes whose (bin) is assigned to this core at block ids//128
        for b_i in range(nb):
            # find global bin g with core_of_bin[g]==c and block_of_bin[g]==b_i
            g = np.where((core_of_bin == c) & (block_of_bin == b_i))[0][0]
            sel = bin_of == g
            nds = np.where(sel)[0]
            sl = slot_of[nds]
            rows[b_i * 128 + sl] = nft[nds]
            mask[b_i * 128 + sl] = deg[nds] > 0
        nftT_c = rows.T.astype(np.float16).copy()
        nftT_cm = (rows * mask[:, None]).T.astype(np.float16).copy()

        m = {
            "eftT": eftT_c,
            "hsT": hsT_c,
            "nftT_c": nftT_c,
            "nftT_cm": nftT_cm,
            "w2cat": w2cat,
            "w1qa": w1qa,
            "w3": w3_np,
            "Pcat": Pcat_c,
            "rdst": rdst_cat,
        }
        if has_bias:
            m["brow"] = b.astype(np.float16).reshape(1, F)
        in_maps.append(m)

    unperm = (core_of_bin, block_of_bin, bin_of, slot_of)
    return in_maps, unperm, tpb, has_bias


_NC_CACHE = {}


def _get_nc(key, *args, **kw):
    if key not in _NC_CACHE:
        _NC_CACHE[key] = build_nc(*args, **kw)
    return _NC_CACHE[key]


def run(nft, eft, W_path, b_path, W_attn1, attn2, src, dst, trace=False,
        tmpdir=None, prec="f16"):
    n_nodes = nft.shape[0]
    npc = ((n_nodes + NCORES - 1) // NCORES + 127) // 128 * 128
    nb = npc // 128

    in_maps, unperm, tpb, has_bias = prep_inputs(
        np.asarray(nft), np.asarray(eft), np.asarray(W_path),
        np.asarray(b_path), np.asarray(W_attn1), np.asarray(attn2),
        np.asarray(src), np.asarray(dst), npc)
    core_of_bin, block_of_bin, bin_of, slot_of = unperm
    npad = NCORES * nb * 128

    nc = _get_nc((npc, tpb, npad, has_bias), npc, tpb, npad, has_bias)
    kw = {}
    if trace:
        kw = dict(trace=True, tmpdir=tmpdir)
    res = bass_utils.run_bass_kernel_spmd(nc, in_maps,
                                          core_ids=list(range(NCORES)), **kw)

    out = np.empty((n_nodes, F), dtype=np.float32)
    cols = block_of_bin[bin_of] * 128 + slot_of  # column in owning core's outT
    cores = core_of_bin[bin_of]
    for c in range(NCORES):
        sel = cores == c
        out[sel] = res.results[c]["outT"][:, cols[sel]].T
    return out, res


def kernel(**inputs):
    out, _ = run(**inputs)
    return out


# revision 47
# speedup vs baseline: 1.8640x; 1.0758x over previous
"""GAT message-passing kernel for 8 Trainium2 NeuronCores (Bass/Tile).

Strategy ("route edges by dst ownership", no collectives):
  - Host renumbers nodes (LPT bin-packing by degree) into 160 blocks of <=128
    nodes so every 128-node block has nearly equal in-edge count; each core
    owns 20 blocks, so segment-softmax and scatter-sum are fully core-local.
  - Reassociation: epaths = y1[src] + eft@W2 + y3[dst] + b with y1 = nft@W1.
    Since sum(att)=1 per (node, head), the y3[dst] part of the aggregated
    message is exactly +y3[dst], added once per node in phase 3.  Softmax is
    computed without max-subtraction; a fixed shift exp(a-7) keeps the
    unnormalized weights in fp16 range (softmax is shift-invariant).
  - No node-table phase and no device-side gather: the src node features are
    gathered per edge on the HOST (the sharding hint: "each device holds the
    gathered src/dst features") into a dense transposed tensor hsT that the
    kernel streams with full-rate 2KB DMA descriptors, and y1/qa are computed
    per edge on the PE:
      psEP[:, k, 0:136] = eftT_tile.T @ [W2 | W2A2] + hsT_tile.T @ [W1 | Wqa]
    where Wqa = W_attn1 + W1@A2. The r[dst] = y3@A2 logit term is tiny
    (8 floats per node) and is gathered per edge on the host (rdst); dummy
    padding edges get rdst = -1e4 which drives their weight u to exp(<-100)=0.
  - Per tile: one-hot P (dst-slot x edge) is built on-chip by a DVE is_equal
    against an iota row; P is the stationary of the scatter matmul that
    accumulates [agg | s] for the owning 128-node block.
  - Phase 3 (per node block): agg/s, transpose via PE, += nft@W3 (matmul
    accumulate, deg>0-masked) and += nft, relu, store transposed output.
"""

import sys
import heapq
import numpy as np
import ml_dtypes

for _p in ("/opt/trn_rl_repo",):
    if _p not in sys.path:
        sys.path.append(_p)

import concourse.bacc as bacc
import concourse.bass as bass
import concourse.mybir as mybir
from concourse.tile import TileContext
from concourse import bass_utils

F = 128
H = 8
DH = 16
F2 = F + H  # 136
NCORES = 8
EXP_SHIFT = 7.0  # exp(a - shift); softmax-invariant, keeps u in fp16 range
BATCH = 3        # tiles per PSUM epilogue batch ([128, 3, 136] f32 <= 2KB bank)


def build_nc(npc, tpb, npad, has_bias, debug=False):
    nb = npc // 128                  # node blocks per core
    ntiles = nb * tpb                # edge tiles per core
    epad = ntiles * 128              # padded edge count per core
    dt = mybir.dt
    AOP = mybir.AluOpType

    nc = bacc.Bacc("TRN2", target_bir_lowering=False, debug=False,
                   num_devices=NCORES, num_swdge_queues=4)

    # ---- inputs ----
    eftT = nc.dram_tensor("eftT", (F, epad), dt.float16, kind="ExternalInput")
    hsT_in = nc.dram_tensor("hsT", (F, epad), dt.float16, kind="ExternalInput")
    nftT_c = nc.dram_tensor("nftT_c", (F, npc), dt.float16, kind="ExternalInput")
    nftT_cm = nc.dram_tensor("nftT_cm", (F, npc), dt.float16, kind="ExternalInput")
    w2cat_in = nc.dram_tensor("w2cat", (F, F2), dt.float16, kind="ExternalInput")
    w1qa_in = nc.dram_tensor("w1qa", (F, F2), dt.float16, kind="ExternalInput")
    w3_in = nc.dram_tensor("w3", (F, F), dt.float16, kind="ExternalInput")
    Pcat = nc.dram_tensor("Pcat", (128, epad), dt.float8e4, kind="ExternalInput")
    rdst_in = nc.dram_tensor("rdst", (128, ntiles * H), dt.float16, kind="ExternalInput")
    if has_bias:
        brow_in = nc.dram_tensor("brow", (1, F), dt.float16, kind="ExternalInput")

    outT = nc.dram_tensor("outT", (F, npc), dt.float32, kind="ExternalOutput")

    with TileContext(nc) as tc:
        with tc.tile_pool(name="const", bufs=1) as cpool, \
             tc.tile_pool(name="work", bufs=3) as pool, \
             tc.tile_pool(name="io", bufs=6) as iop, \
             tc.tile_pool(name="psEP", bufs=4, space="PSUM") as psEP, \
             tc.tile_pool(name="psB", bufs=2, space="PSUM") as psB, \
             tc.tile_pool(name="psC", bufs=2, space="PSUM") as psC:

            # ---------- constants ----------
            iota_row = cpool.tile([128, 128], dt.float32)
            nc.gpsimd.iota(iota_row, pattern=[[1, 128]], channel_multiplier=0,
                           allow_small_or_imprecise_dtypes=True)
            iota_col = cpool.tile([128, 1], dt.float32)
            nc.gpsimd.iota(iota_col, pattern=[[1, 1]], channel_multiplier=1,
                           allow_small_or_imprecise_dtypes=True)
            ident = cpool.tile([128, 128], dt.float32)
            nc.vector.tensor_scalar(out=ident, in0=iota_row[:, :],
                                    scalar1=iota_col[:, :], scalar2=None,
                                    op0=AOP.is_equal)
            nshift = cpool.tile([128, 1], dt.float32)
            nc.vector.memset(nshift, -EXP_SHIFT)
            ident16 = cpool.tile([128, 128], dt.float16)
            nc.vector.tensor_copy(out=ident16, in_=ident)

            w2cat_s = cpool.tile([F, F2], dt.float16)
            nc.sync.dma_start(out=w2cat_s, in_=w2cat_in[:, :])
            w1qa_s = cpool.tile([F, F2], dt.float16)
            nc.sync.dma_start(out=w1qa_s, in_=w1qa_in[:, :])
            w3_s = cpool.tile([F, F], dt.float16)
            nc.sync.dma_start(out=w3_s, in_=w3_in[:, :])

            if has_bias:
                brow_s = cpool.tile([1, F], dt.float16)
                nc.sync.dma_start(out=brow_s, in_=brow_in[:, :])
                ones_row = cpool.tile([1, 128], dt.float16)
                nc.vector.memset(ones_row, 1.0)

            rdst_s = cpool.tile([128, ntiles * H], dt.float16)
            nc.sync.dma_start(out=rdst_s, in_=rdst_in[:, :])
            nftc_s = cpool.tile([128, npc], dt.float16)
            nc.sync.dma_start(out=nftc_s, in_=nftT_c[:, :])
            nftcm_s = cpool.tile([128, npc], dt.float16)
            nc.sync.dma_start(out=nftcm_s, in_=nftT_cm[:, :])

            # ---------- main loop over edge tiles ----------
            psb_cur = None
            eft_ld = hsT = None
            psa = None
            CH = 32  # tiles per DMA chunk (8KB per partition per stream)
            pch_hist = {}
            for t in range(ntiles):
                b, j = divmod(t, tpb)
                tB = t % BATCH
                tc16 = t % CH
                if tc16 == 0:
                    w = min(CH * 128, (ntiles - t) * 128)
                    nch16 = w // 128
                    eft_ld = iop.tile([128, CH * 128], dt.float16, tag="eft")
                    nc.sync.dma_start(out=eft_ld[:, 0:w],
                                      in_=eftT[:, t * 128:t * 128 + w])
                    hsT = iop.tile([128, CH * 128], dt.float16, tag="hsT")
                    nc.sync.dma_start(out=hsT[:, 0:w],
                                      in_=hsT_in[:, t * 128:t * 128 + w])
                    # host-built one-hot P (fp8), streamed like eft/hs
                    pchunk = iop.tile([128, CH * 128], dt.float8e4, tag="pchunk")
                    nc.sync.dma_start(out=pchunk[:, 0:w],
                                      in_=Pcat[:, t * 128:t * 128 + w])
                    pch_hist[t // CH] = pchunk
                    pch_hist.pop(t // CH - 2, None)
                if tB == 0:
                    psa = psEP.tile([128, BATCH, F2], dt.float32, tag="ep")

                et = eft_ld[:, tc16 * 128:(tc16 + 1) * 128]
                hst = hsT[:, tc16 * 128:(tc16 + 1) * 128]
                nc.tensor.matmul(psa[:, tB, :], lhsT=et, rhs=w2cat_s,
                                 start=True, stop=False, skip_group_check=True)
                nc.tensor.matmul(psa[:, tB, :], lhsT=hst, rhs=w1qa_s,
                                 start=False, stop=not has_bias,
                                 skip_group_check=True)
                if has_bias:
                    nc.tensor.matmul(psa[:, tB, 0:F], lhsT=ones_row, rhs=brow_s,
                                     start=False, stop=True,
                                     skip_group_check=True)

                if tB != (min(BATCH, ntiles - t + tB) - 1):
                    continue
                # ---- batch epilogue: n4 tiles of logits and messages ----
                n4 = tB + 1
                tb = t - tB
                z4 = pool.tile([128, BATCH, H], dt.float32, tag="z4")
                nc.vector.tensor_tensor(
                    out=z4[:, 0:n4, :],
                    in0=psa[:, 0:n4, F:F2],
                    in1=rdst_s[:, tb * H:(tb + n4) * H].rearrange(
                        "p (k h) -> p k h", h=H),
                    op=AOP.add)
                a4 = pool.tile([128, BATCH, H], dt.float32, tag="a4")
                nc.vector.scalar_tensor_tensor(
                    out=a4[:, 0:n4, :], in0=z4[:, 0:n4, :], scalar=0.01,
                    in1=z4[:, 0:n4, :], op0=AOP.mult, op1=AOP.max)
                msgu4 = pool.tile([128, BATCH, F2], dt.float16, tag="msgu4")
                nc.scalar.activation(
                    msgu4[:, 0:n4, F:F2], a4[:, 0:n4, :],
                    mybir.ActivationFunctionType.Exp,
                    bias=nshift[:, :])
                nc.vector.tensor_tensor(
                    out=msgu4[:, 0:n4, 0:F].rearrange("p k (h d) -> p k h d", h=H),
                    in0=psa[:, 0:n4, 0:F].rearrange("p k (h d) -> p k h d", h=H),
                    in1=msgu4[:, 0:n4, F:F2][:, :, :, None]
                        .broadcast_to((128, n4, H, DH)),
                    op=AOP.mult)

                # scatter each tile of the batch into its block accumulator
                for k in range(n4):
                    tg = tb + k
                    bb, jj = divmod(tg, tpb)
                    pk = pch_hist[tg // CH][:, (tg % CH) * 128:
                                            (tg % CH + 1) * 128]
                    if jj == 0:
                        psb_cur = psB.tile([128, F2], dt.float32, tag="aggB")
                    nc.tensor.matmul(psb_cur, lhsT=pk, rhs=msgu4[:, k, :],
                                     start=(jj == 0), stop=(jj == tpb - 1),
                                     skip_group_check=True)
                    if jj != tpb - 1:
                        continue
                    # ---------- phase 3 for block bb ----------
                    ss = pool.tile([128, H], dt.float32, tag="ss")
                    nc.vector.tensor_scalar(out=ss, in0=psb_cur[:, F:F2],
                                            scalar1=1e-30, scalar2=None,
                                            op0=AOP.max)
                    inv = pool.tile([128, H], dt.float32, tag="inv")
                    nc.vector.reciprocal(inv, ss)
                    mn = pool.tile([128, F], dt.float32, tag="mn")
                    nc.vector.tensor_tensor(
                        out=mn[:, :].rearrange("p (h d) -> p h d", h=H),
                        in0=psb_cur[:, 0:F].rearrange("p (h d) -> p h d", h=H),
                        in1=inv[:, :, None].broadcast_to((128, H, DH)),
                        op=AOP.mult)
                    psc = psC.tile([128, 128], dt.float32, tag="outC")
                    nc.tensor.matmul(psc, lhsT=w3_s,
                                     rhs=nftcm_s[:, bb * 128:(bb + 1) * 128],
                                     start=True, stop=False)
                    nc.tensor.matmul(psc, lhsT=mn, rhs=ident,
                                     is_transpose=True,
                                     start=False, stop=False)
                    nc.tensor.matmul(psc, lhsT=ident16,
                                     rhs=nftc_s[:, bb * 128:(bb + 1) * 128],
                                     start=False, stop=True)
                    oc2 = pool.tile([128, 128], dt.float32, tag="oc2")
                    nc.scalar.activation(oc2, psc,
                                         mybir.ActivationFunctionType.Relu)
                    nc.sync.dma_start(out=outT[:, bb * 128:(bb + 1) * 128],
                                      in_=oc2)

    nc.compile()
    return nc


def _binpack(deg, nbins, cap):
    """LPT bin-packing of nodes into nbins bins of <= cap nodes, minimizing
    the max per-bin degree sum. Returns (bin_of_node, slot_of_node, loads)."""
    n = len(deg)
    order = np.argsort(-deg, kind="stable")
    bin_of = np.empty(n, dtype=np.int64)
    slot_of = np.empty(n, dtype=np.int64)
    counts = np.zeros(nbins, dtype=np.int64)
    loads = np.zeros(nbins, dtype=np.int64)
    heap = [(0, i) for i in range(nbins)]
    heapq.heapify(heap)
    for nd in order:
        while True:
            load, b = heapq.heappop(heap)
            if counts[b] < cap:
                break
        bin_of[nd] = b
        slot_of[nd] = counts[b]
        counts[b] += 1
        loads[b] += deg[nd]
        if counts[b] < cap:
            heapq.heappush(heap, (loads[b], b))
    return bin_of, slot_of, loads


def prep_inputs(nft, eft, W_path, b_path, W_attn1, attn2, src, dst, npc):
    """Host-side sharding/relayout. Returns (in_maps, unperm, tpb, has_bias)."""
    n_nodes = nft.shape[0]
    nb = npc // 128
    nbins = NCORES * nb

    nft = np.ascontiguousarray(nft, dtype=np.float32)
    src = np.asarray(src, dtype=np.int64)
    dst = np.asarray(dst, dtype=np.int64)
    deg = np.bincount(dst, minlength=n_nodes)

    # --- node renumbering: balance per-block edge counts ---
    bin_of, slot_of, loads = _binpack(deg, nbins, 128)
    tpb = max(1, int(np.ceil(loads.max() / 128.0)))
    # assign bins to cores (LPT on loads, 20 bins per core)
    order = np.argsort(-loads, kind="stable")
    core_of_bin = np.empty(nbins, dtype=np.int64)
    block_of_bin = np.empty(nbins, dtype=np.int64)
    cheap = [(0, c) for c in range(NCORES)]
    heapq.heapify(cheap)
    ccount = np.zeros(NCORES, dtype=np.int64)
    for g in order:
        while True:
            load, c = heapq.heappop(cheap)
            if ccount[c] < nb:
                break
        core_of_bin[g] = c
        block_of_bin[g] = ccount[c]
        ccount[c] += 1
        load += loads[g]
        if ccount[c] < nb:
            heapq.heappush(cheap, (load, c))

    ntiles = nb * tpb
    epad = ntiles * 128
    npad = nbins * 128

    nft16 = nft.astype(np.float16)

    # attention combination weights
    a2 = np.asarray(attn2, dtype=np.float32).reshape(H, DH)
    A2blk = np.zeros((F, H), dtype=np.float32)
    for h in range(H):
        A2blk[h * DH:(h + 1) * DH, h] = a2[h]
    Wp = np.ascontiguousarray(W_path, dtype=np.float32)
    W1, W2, W3 = Wp[0:F], Wp[F:2 * F], Wp[2 * F:3 * F]
    w2cat = np.concatenate([W2, W2 @ A2blk], axis=1).astype(np.float16)
    w1qa = np.concatenate(
        [W1, np.asarray(W_attn1, np.float32) + W1 @ A2blk], axis=1
    ).astype(np.float16)
    w3_np = W3.astype(np.float16)

    has_bias = bool(np.any(np.asarray(b_path) != 0))
    b = np.asarray(b_path, dtype=np.float32).reshape(F)
    bqa = b @ A2blk  # folded into rdst
    # r[dst] logit term (y3@A2): tiny per-node table, gathered on host
    r_node = (nft @ (W3 @ A2blk)).astype(np.float32)  # [N, H]

    # --- edge routing ---
    core_of_e = core_of_bin[bin_of[dst]]
    sortkey = (core_of_e * nb + block_of_bin[bin_of[dst]]) * 128 + slot_of[dst]
    perm = np.argsort(sortkey, kind="stable")
    s_key = sortkey[perm]
    s_src = src[perm]
    s_dst = dst[perm]
    s_eft = np.asarray(eft)[perm]

    in_maps = []
    blkkey = s_key // 128  # global (core*nb + block) of each sorted edge
    for c in range(NCORES):
        eftT_c = np.zeros((F, epad), dtype=np.float16)
        hsT_c = np.zeros((F, epad), dtype=np.float16)
        dstloc = np.zeros(epad, dtype=np.int64)
        rdst = np.full((epad, H), -10000.0, dtype=np.float32)

        for b_i in range(nb):
            gk = c * nb + b_i
            s = np.searchsorted(blkkey, gk)
            e = np.searchsorted(blkkey, gk + 1)
            cnt = e - s
            assert cnt <= tpb * 128, f"block overflow: {cnt} > {tpb * 128}"
            o = b_i * tpb * 128
            eftT_c[:, o:o + cnt] = s_eft[s:e].T.astype(np.float16)
            hsT_c[:, o:o + cnt] = nft16[s_src[s:e]].T
            dstloc[o:o + cnt] = slot_of[s_dst[s:e]]
            rdst[o:o + cnt] = r_node[s_dst[s:e]] + bqa

        ee = np.arange(epad)
        Pcat_c = np.zeros((128, epad), dtype=mybir.dt.np(mybir.dt.float8e4))
        Pcat_c[ee % 128, (ee // 128) * 128 + dstloc] = 1.0
        rdst_cat = np.ascontiguousarray(
            rdst.reshape(ntiles, 128, H).transpose(1, 0, 2).reshape(128, ntiles * H)
        ).astype(np.float16)

        # per-core node features (new order)
        ids = np.arange(npc, dtype=np.int64)
        gbin = c * nb + ids // 128
        rows = np.zeros((npc, F), dtype=np.float32)
        mask = np.zeros(npc, dtype=bool)
        # nodes whose (bin) is assigned to this core at block ids//128
        for b_i in range(nb):
            # find global bin g with core_of_bin[g]==c and block_of_bin[g]==b_i
            g = np.where((core_of_bin == c) & (block_of_bin == b_i))[0][0]
            sel = bin_of == g
            nds = np.where(sel)[0]
            sl = slot_of[nds]
            rows[b_i * 128 + sl] = nft[nds]
            mask[b_i * 128 + sl] = deg[nds] > 0
        nftT_c = rows.T.astype(np.float16).copy()
        nftT_cm = (rows * mask[:, None]).T.astype(np.float16).copy()

        m = {
            "eftT": eftT_c,
            "hsT": hsT_c,
            "nftT_c": nftT_c,
            "nftT_cm": nftT_cm,
            "w2cat": w2cat,
            "w1qa": w1qa,
            "w3": w3_np,
            "Pcat": Pcat_c,
            "rdst": rdst_cat,
        }
        if has_bias:
            m["brow"] = b.astype(np.float16).reshape(1, F)
        in_maps.append(m)

    unperm = (core_of_bin, block_of_bin, bin_of, slot_of)
    return in_maps, unperm, tpb, has_bias


_NC_CACHE = {}


def _get_nc(key, *args, **kw):
    if key not in _NC_CACHE:
        _NC_CACHE[key] = build_nc(*args, **kw)
    return _NC_CACHE[key]


def run(nft, eft, W_path, b_path, W_attn1, attn2, src, dst, trace=False,
        tmpdir=None, prec="f16"):
    n_nodes = nft.shape[0]
    npc = ((n_nodes + NCORES - 1) // NCORES + 127) // 128 * 128
    nb = npc // 128

    in_maps, unperm, tpb, has_bias = prep_inputs(
        np.asarray(nft), np.asarray(eft), np.asarray(W_path),
        np.asarray(b_path), np.asarray(W_attn1), np.asarray(attn2),
        np.asarray(src), np.asarray(dst), npc)
    core_of_bin, block_of_bin, bin_of, slot_of = unperm
    npad = NCORES * nb * 128

    nc = _get_nc((npc, tpb, npad, has_bias), npc, tpb, npad, has_bias)
    kw = {}
    if trace:
        kw = dict(trace=True, tmpdir=tmpdir)
    res = bass_utils.run_bass_kernel_spmd(nc, in_maps,
                                          core_ids=list(range(NCORES)), **kw)

    out = np.empty((n_nodes, F), dtype=np.float32)
    cols = block_of_bin[bin_of] * 128 + slot_of  # column in owning core's outT
    cores = core_of_bin[bin_of]
    for c in range(NCORES):
        sel = cores == c
        out[sel] = res.results[c]["outT"][:, cols[sel]].T
    return out, res


def kernel(**inputs):
    out, _ = run(**inputs)
    return out


# revision 53
# speedup vs baseline: 1.9656x; 1.0545x over previous
"""GAT message-passing kernel for 8 Trainium2 NeuronCores (Bass/Tile).

Strategy ("route edges by dst ownership", no collectives):
  - Host renumbers nodes (LPT bin-packing by degree) into 160 blocks of <=128
    nodes so every 128-node block has nearly equal in-edge count; each core
    owns 20 blocks, so segment-softmax and scatter-sum are fully core-local.
  - Reassociation: epaths = y1[src] + eft@W2 + y3[dst] + b with y1 = nft@W1.
    Since sum(att)=1 per (node, head), the y3[dst] part of the aggregated
    message is exactly +y3[dst], added once per node in phase 3.  Softmax is
    computed without max-subtraction; a fixed shift exp(a-7) keeps the
    unnormalized weights in fp16 range (softmax is shift-invariant).
  - No node-table phase and no device-side gather: the src node features are
    gathered per edge on the HOST (the sharding hint: "each device holds the
    gathered src/dst features") into a dense transposed tensor hsT that the
    kernel streams with full-rate 2KB DMA descriptors, and y1/qa are computed
    per edge on the PE:
      psEP[:, k, 0:136] = eftT_tile.T @ [W2 | W2A2] + hsT_tile.T @ [W1 | Wqa]
    where Wqa = W_attn1 + W1@A2. The r[dst] = y3@A2 logit term is tiny
    (8 floats per node) and is gathered per edge on the host (rdst); dummy
    padding edges get rdst = -1e4 which drives their weight u to exp(<-100)=0.
  - Per tile: one-hot P (dst-slot x edge) is built on-chip by a DVE is_equal
    against an iota row; P is the stationary of the scatter matmul that
    accumulates [agg | s] for the owning 128-node block.
  - Phase 3 (per node block): agg/s, transpose via PE, += nft@W3 (matmul
    accumulate, deg>0-masked) and += nft, relu, store transposed output.
"""

import sys
import heapq
import numpy as np
import ml_dtypes

for _p in ("/opt/trn_rl_repo",):
    if _p not in sys.path:
        sys.path.append(_p)

import concourse.bacc as bacc
import concourse.bass as bass
import concourse.mybir as mybir
from concourse.tile import TileContext
from concourse import bass_utils

F = 128
H = 8
DH = 16
F2 = F + H  # 136
NCORES = 8
EXP_SHIFT = 7.0  # exp(a - shift); softmax-invariant, keeps u in fp16 range
BATCH = 3        # tiles per PSUM epilogue batch ([128, 3, 136] f32 <= 2KB bank)


def build_nc(npc, tpb, npad, has_bias, debug=False):
    nb = npc // 128                  # node blocks per core
    ntiles = nb * tpb                # edge tiles per core
    epad = ntiles * 128              # padded edge count per core
    dt = mybir.dt
    AOP = mybir.AluOpType

    nc = bacc.Bacc("TRN2", target_bir_lowering=False, debug=False,
                   num_devices=NCORES, num_swdge_queues=4)

    # ---- inputs ----
    eftT = nc.dram_tensor("eftT", (F, epad), dt.float8e3, kind="ExternalInput")
    hsT_in = nc.dram_tensor("hsT", (F, epad), dt.float16, kind="ExternalInput")
    nftT_c = nc.dram_tensor("nftT_c", (F, npc), dt.float16, kind="ExternalInput")
    nftT_cm = nc.dram_tensor("nftT_cm", (F, npc), dt.float16, kind="ExternalInput")
    w2cat_in = nc.dram_tensor("w2cat", (F, F2), dt.float16, kind="ExternalInput")
    w1qa_in = nc.dram_tensor("w1qa", (F, F2), dt.float16, kind="ExternalInput")
    w3_in = nc.dram_tensor("w3", (F, F), dt.float16, kind="ExternalInput")
    Pcat = nc.dram_tensor("Pcat", (128, epad), dt.float8e4, kind="ExternalInput")
    rdst_in = nc.dram_tensor("rdst", (128, ntiles * H), dt.float16, kind="ExternalInput")
    if has_bias:
        brow_in = nc.dram_tensor("brow", (1, F), dt.float16, kind="ExternalInput")

    outT = nc.dram_tensor("outT", (F, npc), dt.float16, kind="ExternalOutput")

    with TileContext(nc) as tc:
        with tc.tile_pool(name="const", bufs=1) as cpool, \
             tc.tile_pool(name="work", bufs=3) as pool, \
             tc.tile_pool(name="io", bufs=6) as iop, \
             tc.tile_pool(name="psEP", bufs=4, space="PSUM") as psEP, \
             tc.tile_pool(name="psB", bufs=2, space="PSUM") as psB, \
             tc.tile_pool(name="psC", bufs=2, space="PSUM") as psC:

            # ---------- constants ----------
            iota_row = cpool.tile([128, 128], dt.float32)
            nc.gpsimd.iota(iota_row, pattern=[[1, 128]], channel_multiplier=0,
                           allow_small_or_imprecise_dtypes=True)
            iota_col = cpool.tile([128, 1], dt.float32)
            nc.gpsimd.iota(iota_col, pattern=[[1, 1]], channel_multiplier=1,
                           allow_small_or_imprecise_dtypes=True)
            ident = cpool.tile([128, 128], dt.float32)
            nc.vector.tensor_scalar(out=ident, in0=iota_row[:, :],
                                    scalar1=iota_col[:, :], scalar2=None,
                                    op0=AOP.is_equal)
            nshift = cpool.tile([128, 1], dt.float32)
            nc.vector.memset(nshift, -EXP_SHIFT)
            ident16 = cpool.tile([128, 128], dt.float16)
            nc.vector.tensor_copy(out=ident16, in_=ident)

            w2cat_s = cpool.tile([F, F2], dt.float16)
            nc.sync.dma_start(out=w2cat_s, in_=w2cat_in[:, :])
            w1qa_s = cpool.tile([F, F2], dt.float16)
            nc.sync.dma_start(out=w1qa_s, in_=w1qa_in[:, :])
            w3_s = cpool.tile([F, F], dt.float16)
            nc.sync.dma_start(out=w3_s, in_=w3_in[:, :])

            if has_bias:
                brow_s = cpool.tile([1, F], dt.float16)
                nc.sync.dma_start(out=brow_s, in_=brow_in[:, :])
                ones_row = cpool.tile([1, 128], dt.float16)
                nc.vector.memset(ones_row, 1.0)

            rdst_s = cpool.tile([128, ntiles * H], dt.float16)
            nc.sync.dma_start(out=rdst_s, in_=rdst_in[:, :])
            nftc_s = cpool.tile([128, npc], dt.float16)
            nc.sync.dma_start(out=nftc_s, in_=nftT_c[:, :])
            nftcm_s = cpool.tile([128, npc], dt.float16)
            nc.sync.dma_start(out=nftcm_s, in_=nftT_cm[:, :])

            # ---------- main loop over edge tiles ----------
            psb_cur = None
            eft_ld = hsT = None
            psa = None
            CH = 32  # tiles per DMA chunk (8KB per partition per stream)
            pch_hist = {}
            for t in range(ntiles):
                b, j = divmod(t, tpb)
                tB = t % BATCH
                tc16 = t % CH
                if tc16 == 0:
                    w = min(CH * 128, (ntiles - t) * 128)
                    nch16 = w // 128
                    eft_ld = iop.tile([128, CH * 128], dt.float8e3, tag="eft")
                    nc.sync.dma_start(out=eft_ld[:, 0:w],
                                      in_=eftT[:, t * 128:t * 128 + w])
                    hsT = iop.tile([128, CH * 128], dt.float16, tag="hsT")
                    nc.sync.dma_start(out=hsT[:, 0:w],
                                      in_=hsT_in[:, t * 128:t * 128 + w])
                    # host-built one-hot P (fp8), streamed like eft/hs
                    pchunk = iop.tile([128, CH * 128], dt.float8e4, tag="pchunk")
                    nc.sync.dma_start(out=pchunk[:, 0:w],
                                      in_=Pcat[:, t * 128:t * 128 + w])
                    pch_hist[t // CH] = pchunk
                    pch_hist.pop(t // CH - 2, None)
                if tB == 0:
                    psa = psEP.tile([128, BATCH, F2], dt.float32, tag="ep")

                et = eft_ld[:, tc16 * 128:(tc16 + 1) * 128]
                hst = hsT[:, tc16 * 128:(tc16 + 1) * 128]
                nc.tensor.matmul(psa[:, tB, :], lhsT=et, rhs=w2cat_s,
                                 start=True, stop=False, skip_group_check=True)
                nc.tensor.matmul(psa[:, tB, :], lhsT=hst, rhs=w1qa_s,
                                 start=False, stop=not has_bias,
                                 skip_group_check=True)
                if has_bias:
                    nc.tensor.matmul(psa[:, tB, 0:F], lhsT=ones_row, rhs=brow_s,
                                     start=False, stop=True,
                                     skip_group_check=True)

                if tB != (min(BATCH, ntiles - t + tB) - 1):
                    continue
                # ---- batch epilogue: n4 tiles of logits and messages ----
                n4 = tB + 1
                tb = t - tB
                z4 = pool.tile([128, BATCH, H], dt.float32, tag="z4")
                nc.vector.tensor_tensor(
                    out=z4[:, 0:n4, :],
                    in0=psa[:, 0:n4, F:F2],
                    in1=rdst_s[:, tb * H:(tb + n4) * H].rearrange(
                        "p (k h) -> p k h", h=H),
                    op=AOP.add)
                a4 = pool.tile([128, BATCH, H], dt.float32, tag="a4")
                nc.vector.scalar_tensor_tensor(
                    out=a4[:, 0:n4, :], in0=z4[:, 0:n4, :], scalar=0.01,
                    in1=z4[:, 0:n4, :], op0=AOP.mult, op1=AOP.max)
                msgu4 = pool.tile([128, BATCH, F2], dt.float16, tag="msgu4")
                nc.scalar.activation(
                    msgu4[:, 0:n4, F:F2], a4[:, 0:n4, :],
                    mybir.ActivationFunctionType.Exp,
                    bias=nshift[:, :])
                nc.vector.tensor_tensor(
                    out=msgu4[:, 0:n4, 0:F].rearrange("p k (h d) -> p k h d", h=H),
                    in0=psa[:, 0:n4, 0:F].rearrange("p k (h d) -> p k h d", h=H),
                    in1=msgu4[:, 0:n4, F:F2][:, :, :, None]
                        .broadcast_to((128, n4, H, DH)),
                    op=AOP.mult)

                # scatter each tile of the batch into its block accumulator
                for k in range(n4):
                    tg = tb + k
                    bb, jj = divmod(tg, tpb)
                    pk = pch_hist[tg // CH][:, (tg % CH) * 128:
                                            (tg % CH + 1) * 128]
                    if jj == 0:
                        psb_cur = psB.tile([128, F2], dt.float32, tag="aggB")
                    nc.tensor.matmul(psb_cur, lhsT=pk, rhs=msgu4[:, k, :],
                                     start=(jj == 0), stop=(jj == tpb - 1),
                                     skip_group_check=True)
                    if jj != tpb - 1:
                        continue
                    # ---------- phase 3 for block bb ----------
                    ss = pool.tile([128, H], dt.float32, tag="ss")
                    nc.vector.tensor_scalar(out=ss, in0=psb_cur[:, F:F2],
                                            scalar1=1e-30, scalar2=None,
                                            op0=AOP.max)
                    inv = pool.tile([128, H], dt.float32, tag="inv")
                    nc.vector.reciprocal(inv, ss)
                    mn = pool.tile([128, F], dt.float32, tag="mn")
                    nc.vector.tensor_tensor(
                        out=mn[:, :].rearrange("p (h d) -> p h d", h=H),
                        in0=psb_cur[:, 0:F].rearrange("p (h d) -> p h d", h=H),
                        in1=inv[:, :, None].broadcast_to((128, H, DH)),
                        op=AOP.mult)
                    psc = psC.tile([128, 128], dt.float32, tag="outC")
                    nc.tensor.matmul(psc, lhsT=w3_s,
                                     rhs=nftcm_s[:, bb * 128:(bb + 1) * 128],
                                     start=True, stop=False)
                    nc.tensor.matmul(psc, lhsT=mn, rhs=ident,
                                     is_transpose=True,
                                     start=False, stop=False)
                    nc.tensor.matmul(psc, lhsT=ident16,
                                     rhs=nftc_s[:, bb * 128:(bb + 1) * 128],
                                     start=False, stop=True)
                    oc2 = pool.tile([128, 128], dt.float16, tag="oc2")
                    nc.scalar.activation(oc2, psc,
                                         mybir.ActivationFunctionType.Relu)
                    nc.sync.dma_start(out=outT[:, bb * 128:(bb + 1) * 128],
                                      in_=oc2)

    nc.compile()
    return nc


def _binpack(deg, nbins, cap):
    """LPT bin-packing of nodes into nbins bins of <= cap nodes, minimizing
    the max per-bin degree sum. Returns (bin_of_node, slot_of_node, loads)."""
    n = len(deg)
    order = np.argsort(-deg, kind="stable")
    bin_of = np.empty(n, dtype=np.int64)
    slot_of = np.empty(n, dtype=np.int64)
    counts = np.zeros(nbins, dtype=np.int64)
    loads = np.zeros(nbins, dtype=np.int64)
    heap = [(0, i) for i in range(nbins)]
    heapq.heapify(heap)
    for nd in order:
        while True:
            load, b = heapq.heappop(heap)
            if counts[b] < cap:
                break
        bin_of[nd] = b
        slot_of[nd] = counts[b]
        counts[b] += 1
        loads[b] += deg[nd]
        if counts[b] < cap:
            heapq.heappush(heap, (loads[b], b))
    return bin_of, slot_of, loads


def prep_inputs(nft, eft, W_path, b_path, W_attn1, attn2, src, dst, npc):
    """Host-side sharding/relayout. Returns (in_maps, unperm, tpb, has_bias)."""
    n_nodes = nft.shape[0]
    nb = npc // 128
    nbins = NCORES * nb

    nft = np.ascontiguousarray(nft, dtype=np.float32)
    src = np.asarray(src, dtype=np.int64)
    dst = np.asarray(dst, dtype=np.int64)
    deg = np.bincount(dst, minlength=n_nodes)

    # --- node renumbering: balance per-block edge counts ---
    bin_of, slot_of, loads = _binpack(deg, nbins, 128)
    tpb = max(1, int(np.ceil(loads.max() / 128.0)))
    # assign bins to cores (LPT on loads, 20 bins per core)
    order = np.argsort(-loads, kind="stable")
    core_of_bin = np.empty(nbins, dtype=np.int64)
    block_of_bin = np.empty(nbins, dtype=np.int64)
    cheap = [(0, c) for c in range(NCORES)]
    heapq.heapify(cheap)
    ccount = np.zeros(NCORES, dtype=np.int64)
    for g in order:
        while True:
            load, c = heapq.heappop(cheap)
            if ccount[c] < nb:
                break
        core_of_bin[g] = c
        block_of_bin[g] = ccount[c]
        ccount[c] += 1
        load += loads[g]
        if ccount[c] < nb:
            heapq.heappush(cheap, (load, c))

    ntiles = nb * tpb
    epad = ntiles * 128
    npad = nbins * 128

    nft16 = nft.astype(np.float16)

    # attention combination weights
    a2 = np.asarray(attn2, dtype=np.float32).reshape(H, DH)
    A2blk = np.zeros((F, H), dtype=np.float32)
    for h in range(H):
        A2blk[h * DH:(h + 1) * DH, h] = a2[h]
    Wp = np.ascontiguousarray(W_path, dtype=np.float32)
    W1, W2, W3 = Wp[0:F], Wp[F:2 * F], Wp[2 * F:3 * F]
    w2cat = np.concatenate([W2, W2 @ A2blk], axis=1).astype(np.float16)
    w1qa = np.concatenate(
        [W1, np.asarray(W_attn1, np.float32) + W1 @ A2blk], axis=1
    ).astype(np.float16)
    w3_np = W3.astype(np.float16)

    has_bias = bool(np.any(np.asarray(b_path) != 0))
    b = np.asarray(b_path, dtype=np.float32).reshape(F)
    bqa = b @ A2blk  # folded into rdst
    # r[dst] logit term (y3@A2): tiny per-node table, gathered on host
    r_node = (nft @ (W3 @ A2blk)).astype(np.float32)  # [N, H]

    # --- edge routing ---
    core_of_e = core_of_bin[bin_of[dst]]
    sortkey = (core_of_e * nb + block_of_bin[bin_of[dst]]) * 128 + slot_of[dst]
    perm = np.argsort(sortkey, kind="stable")
    s_key = sortkey[perm]
    s_src = src[perm]
    s_dst = dst[perm]
    s_eft = np.asarray(eft)[perm]

    in_maps = []
    blkkey = s_key // 128  # global (core*nb + block) of each sorted edge
    for c in range(NCORES):
        eftT_c = np.zeros((F, epad), dtype=ml_dtypes.float8_e3m4)
        hsT_c = np.zeros((F, epad), dtype=np.float16)
        dstloc = np.zeros(epad, dtype=np.int64)
        rdst = np.full((epad, H), -10000.0, dtype=np.float32)

        for b_i in range(nb):
            gk = c * nb + b_i
            s = np.searchsorted(blkkey, gk)
            e = np.searchsorted(blkkey, gk + 1)
            cnt = e - s
            assert cnt <= tpb * 128, f"block overflow: {cnt} > {tpb * 128}"
            o = b_i * tpb * 128
            eftT_c[:, o:o + cnt] = s_eft[s:e].T.astype(ml_dtypes.float8_e3m4)
            hsT_c[:, o:o + cnt] = nft16[s_src[s:e]].T
            dstloc[o:o + cnt] = slot_of[s_dst[s:e]]
            rdst[o:o + cnt] = r_node[s_dst[s:e]] + bqa

        ee = np.arange(epad)
        Pcat_c = np.zeros((128, epad), dtype=mybir.dt.np(mybir.dt.float8e4))
        Pcat_c[ee % 128, (ee // 128) * 128 + dstloc] = 1.0
        rdst_cat = np.ascontiguousarray(
            rdst.reshape(ntiles, 128, H).transpose(1, 0, 2).reshape(128, ntiles * H)
        ).astype(np.float16)

        # per-core node features (new order)
        ids = np.arange(npc, dtype=np.int64)
        gbin = c * nb + ids // 128
        rows = np.zeros((npc, F), dtype=np.float32)
        mask = np.zeros(npc, dtype=bool)
        # nodes whose (bin) is assigned to this core at block ids//128
        for b_i in range(nb):
            # find global bin g with core_of_bin[g]==c and block_of_bin[g]==b_i
            g = np.where((core_of_bin == c) & (block_of_bin == b_i))[0][0]
            sel = bin_of == g
            nds = np.where(sel)[0]
            sl = slot_of[nds]
            rows[b_i * 128 + sl] = nft[nds]
            mask[b_i * 128 + sl] = deg[nds] > 0
        nftT_c = rows.T.astype(np.float16).copy()
        nftT_cm = (rows * mask[:, None]).T.astype(np.float16).copy()

        m = {
            "eftT": eftT_c,
            "hsT": hsT_c,
            "nftT_c": nftT_c,
            "nftT_cm": nftT_cm,
            "w2cat": w2cat,
            "w1qa": w1qa,
            "w3": w3_np,
            "Pcat": Pcat_c,
            "rdst": rdst_cat,
        }
        if has_bias:
            m["brow"] = b.astype(np.float16).reshape(1, F)
        in_maps.append(m)

    unperm = (core_of_bin, block_of_bin, bin_of, slot_of)
    return in_maps, unperm, tpb, has_bias


_NC_CACHE = {}


def _get_nc(key, *args, **kw):
    if key not in _NC_CACHE:
        _NC_CACHE[key] = build_nc(*args, **kw)
    return _NC_CACHE[key]


def run(nft, eft, W_path, b_path, W_attn1, attn2, src, dst, trace=False,
        tmpdir=None, prec="f16"):
    n_nodes = nft.shape[0]
    npc = ((n_nodes + NCORES - 1) // NCORES + 127) // 128 * 128
    nb = npc // 128

    in_maps, unperm, tpb, has_bias = prep_inputs(
        np.asarray(nft), np.asarray(eft), np.asarray(W_path),
        np.asarray(b_path), np.asarray(W_attn1), np.asarray(attn2),
        np.asarray(src), np.asarray(dst), npc)
    core_of_bin, block_of_bin, bin_of, slot_of = unperm
    npad = NCORES * nb * 128

    nc = _get_nc((npc, tpb, npad, has_bias), npc, tpb, npad, has_bias)
    kw = {}
    if trace:
        kw = dict(trace=True, tmpdir=tmpdir)
    res = bass_utils.run_bass_kernel_spmd(nc, in_maps,
                                          core_ids=list(range(NCORES)), **kw)

    out = np.empty((n_nodes, F), dtype=np.float32)
    cols = block_of_bin[bin_of] * 128 + slot_of  # column in owning core's outT
    cores = core_of_bin[bin_of]
    for c in range(NCORES):
        sel = cores == c
        out[sel] = res.results[c]["outT"][:, cols[sel]].T
    return out, res


def kernel(**inputs):
    out, _ = run(**inputs)
    return out


# revision 54
# speedup vs baseline: 1.9718x; 1.0032x over previous
"""GAT message-passing kernel for 8 Trainium2 NeuronCores (Bass/Tile).

Strategy ("route edges by dst ownership", no collectives):
  - Host renumbers nodes (LPT bin-packing by degree) into 160 blocks of <=128
    nodes so every 128-node block has nearly equal in-edge count; each core
    owns 20 blocks, so segment-softmax and scatter-sum are fully core-local.
  - Reassociation: epaths = y1[src] + eft@W2 + y3[dst] + b with y1 = nft@W1.
    Since sum(att)=1 per (node, head), the y3[dst] part of the aggregated
    message is exactly +y3[dst], added once per node in phase 3.  Softmax is
    computed without max-subtraction; a fixed shift exp(a-7) keeps the
    unnormalized weights in fp16 range (softmax is shift-invariant).
  - No node-table phase and no device-side gather: the src node features are
    gathered per edge on the HOST (the sharding hint: "each device holds the
    gathered src/dst features") into a dense transposed tensor hsT that the
    kernel streams with full-rate 2KB DMA descriptors, and y1/qa are computed
    per edge on the PE:
      psEP[:, k, 0:136] = eftT_tile.T @ [W2 | W2A2] + hsT_tile.T @ [W1 | Wqa]
    where Wqa = W_attn1 + W1@A2. The r[dst] = y3@A2 logit term is tiny
    (8 floats per node) and is gathered per edge on the host (rdst); dummy
    padding edges get rdst = -1e4 which drives their weight u to exp(<-100)=0.
  - Per tile: one-hot P (dst-slot x edge) is built on-chip by a DVE is_equal
    against an iota row; P is the stationary of the scatter matmul that
    accumulates [agg | s] for the owning 128-node block.
  - Phase 3 (per node block): agg/s, transpose via PE, += nft@W3 (matmul
    accumulate, deg>0-masked) and += nft, relu, store transposed output.
"""

import sys
import heapq
import numpy as np
import ml_dtypes

for _p in ("/opt/trn_rl_repo",):
    if _p not in sys.path:
        sys.path.append(_p)

import concourse.bacc as bacc
import concourse.bass as bass
import concourse.mybir as mybir
from concourse.tile import TileContext
from concourse import bass_utils

F = 128
H = 8
DH = 16
F2 = F + H  # 136
NCORES = 8
EXP_SHIFT = 7.0  # exp(a - shift); softmax-invariant, keeps u in fp16 range
BATCH = 3        # tiles per PSUM epilogue batch ([128, 3, 136] f32 <= 2KB bank)


def build_nc(npc, tpb, npad, has_bias, debug=False):
    nb = npc // 128                  # node blocks per core
    ntiles = nb * tpb                # edge tiles per core
    epad = ntiles * 128              # padded edge count per core
    dt = mybir.dt
    AOP = mybir.AluOpType

    nc = bacc.Bacc("TRN2", target_bir_lowering=False, debug=False,
                   num_devices=NCORES, num_swdge_queues=4)

    # ---- inputs ----
    eftT = nc.dram_tensor("eftT", (F, epad), dt.float8e3, kind="ExternalInput")
    hsT_in = nc.dram_tensor("hsT", (F, epad), dt.float16, kind="ExternalInput")
    nftT_c = nc.dram_tensor("nftT_c", (F, npc), dt.float16, kind="ExternalInput")
    nftT_cm = nc.dram_tensor("nftT_cm", (F, npc), dt.float16, kind="ExternalInput")
    w2cat_in = nc.dram_tensor("w2cat", (F, F2), dt.float16, kind="ExternalInput")
    w1qa_in = nc.dram_tensor("w1qa", (F, F2), dt.float16, kind="ExternalInput")
    w3_in = nc.dram_tensor("w3", (F, F), dt.float16, kind="ExternalInput")
    Pcat = nc.dram_tensor("Pcat", (128, epad), dt.float8e4, kind="ExternalInput")
    rdst_in = nc.dram_tensor("rdst", (128, ntiles * H), dt.float16, kind="ExternalInput")
    if has_bias:
        brow_in = nc.dram_tensor("brow", (1, F), dt.float16, kind="ExternalInput")

    outT = nc.dram_tensor("outT", (F, npc), dt.float16, kind="ExternalOutput")

    with TileContext(nc) as tc:
        with tc.tile_pool(name="const", bufs=1) as cpool, \
             tc.tile_pool(name="work", bufs=3) as pool, \
             tc.tile_pool(name="io", bufs=6) as iop, \
             tc.tile_pool(name="psEP", bufs=4, space="PSUM") as psEP, \
             tc.tile_pool(name="psB", bufs=2, space="PSUM") as psB, \
             tc.tile_pool(name="psC", bufs=2, space="PSUM") as psC:

            # ---------- constants ----------
            iota_row = cpool.tile([128, 128], dt.float32)
            nc.gpsimd.iota(iota_row, pattern=[[1, 128]], channel_multiplier=0,
                           allow_small_or_imprecise_dtypes=True)
            iota_col = cpool.tile([128, 1], dt.float32)
            nc.gpsimd.iota(iota_col, pattern=[[1, 1]], channel_multiplier=1,
                           allow_small_or_imprecise_dtypes=True)
            ident = cpool.tile([128, 128], dt.float32)
            nc.vector.tensor_scalar(out=ident, in0=iota_row[:, :],
                                    scalar1=iota_col[:, :], scalar2=None,
                                    op0=AOP.is_equal)
            nshift = cpool.tile([128, 1], dt.float32)
            nc.vector.memset(nshift, -EXP_SHIFT)
            ident16 = cpool.tile([128, 128], dt.float16)
            nc.vector.tensor_copy(out=ident16, in_=ident)

            w2cat_s = cpool.tile([F, F2], dt.float16)
            nc.sync.dma_start(out=w2cat_s, in_=w2cat_in[:, :])
            w1qa_s = cpool.tile([F, F2], dt.float16)
            nc.sync.dma_start(out=w1qa_s, in_=w1qa_in[:, :])
            w3_s = cpool.tile([F, F], dt.float16)
            nc.sync.dma_start(out=w3_s, in_=w3_in[:, :])

            if has_bias:
                brow_s = cpool.tile([1, F], dt.float16)
                nc.sync.dma_start(out=brow_s, in_=brow_in[:, :])
                ones_row = cpool.tile([1, 128], dt.float16)
                nc.vector.memset(ones_row, 1.0)

            # big constant preloads go on the ACT HWDGE ring so they don't
            # serialize ahead of the first stream chunks on the sync ring
            rdst_s = cpool.tile([128, ntiles * H], dt.float16)
            nc.scalar.dma_start(out=rdst_s, in_=rdst_in[:, :])
            nftc_s = cpool.tile([128, npc], dt.float16)
            nc.scalar.dma_start(out=nftc_s, in_=nftT_c[:, :])
            nftcm_s = cpool.tile([128, npc], dt.float16)
            nc.scalar.dma_start(out=nftcm_s, in_=nftT_cm[:, :])

            # ---------- main loop over edge tiles ----------
            psb_cur = None
            eft_ld = hsT = None
            psa = None
            CH = 32  # tiles per DMA chunk (8KB per partition per stream)
            pch_hist = {}
            for t in range(ntiles):
                b, j = divmod(t, tpb)
                tB = t % BATCH
                tc16 = t % CH
                if tc16 == 0:
                    w = min(CH * 128, (ntiles - t) * 128)
                    nch16 = w // 128
                    eft_ld = iop.tile([128, CH * 128], dt.float8e3, tag="eft")
                    nc.sync.dma_start(out=eft_ld[:, 0:w],
                                      in_=eftT[:, t * 128:t * 128 + w])
                    hsT = iop.tile([128, CH * 128], dt.float16, tag="hsT")
                    nc.sync.dma_start(out=hsT[:, 0:w],
                                      in_=hsT_in[:, t * 128:t * 128 + w])
                    # host-built one-hot P (fp8), streamed like eft/hs
                    pchunk = iop.tile([128, CH * 128], dt.float8e4, tag="pchunk")
                    nc.sync.dma_start(out=pchunk[:, 0:w],
                                      in_=Pcat[:, t * 128:t * 128 + w])
                    pch_hist[t // CH] = pchunk
                    pch_hist.pop(t // CH - 2, None)
                if tB == 0:
                    psa = psEP.tile([128, BATCH, F2], dt.float32, tag="ep")

                et = eft_ld[:, tc16 * 128:(tc16 + 1) * 128]
                hst = hsT[:, tc16 * 128:(tc16 + 1) * 128]
                nc.tensor.matmul(psa[:, tB, :], lhsT=et, rhs=w2cat_s,
                                 start=True, stop=False, skip_group_check=True)
                nc.tensor.matmul(psa[:, tB, :], lhsT=hst, rhs=w1qa_s,
                                 start=False, stop=not has_bias,
                                 skip_group_check=True)
                if has_bias:
                    nc.tensor.matmul(psa[:, tB, 0:F], lhsT=ones_row, rhs=brow_s,
                                     start=False, stop=True,
                                     skip_group_check=True)

                if tB != (min(BATCH, ntiles - t + tB) - 1):
                    continue
                # ---- batch epilogue: n4 tiles of logits and messages ----
                n4 = tB + 1
                tb = t - tB
                z4 = pool.tile([128, BATCH, H], dt.float32, tag="z4")
                nc.vector.tensor_tensor(
                    out=z4[:, 0:n4, :],
                    in0=psa[:, 0:n4, F:F2],
                    in1=rdst_s[:, tb * H:(tb + n4) * H].rearrange(
                        "p (k h) -> p k h", h=H),
                    op=AOP.add)
                a4 = pool.tile([128, BATCH, H], dt.float32, tag="a4")
                nc.vector.scalar_tensor_tensor(
                    out=a4[:, 0:n4, :], in0=z4[:, 0:n4, :], scalar=0.01,
                    in1=z4[:, 0:n4, :], op0=AOP.mult, op1=AOP.max)
                msgu4 = pool.tile([128, BATCH, F2], dt.float16, tag="msgu4")
                nc.scalar.activation(
                    msgu4[:, 0:n4, F:F2], a4[:, 0:n4, :],
                    mybir.ActivationFunctionType.Exp,
                    bias=nshift[:, :])
                nc.vector.tensor_tensor(
                    out=msgu4[:, 0:n4, 0:F].rearrange("p k (h d) -> p k h d", h=H),
                    in0=psa[:, 0:n4, 0:F].rearrange("p k (h d) -> p k h d", h=H),
                    in1=msgu4[:, 0:n4, F:F2][:, :, :, None]
                        .broadcast_to((128, n4, H, DH)),
                    op=AOP.mult)

                # scatter each tile of the batch into its block accumulator
                for k in range(n4):
                    tg = tb + k
                    bb, jj = divmod(tg, tpb)
                    pk = pch_hist[tg // CH][:, (tg % CH) * 128:
                                            (tg % CH + 1) * 128]
                    if jj == 0:
                        psb_cur = psB.tile([128, F2], dt.float32, tag="aggB")
                    nc.tensor.matmul(psb_cur, lhsT=pk, rhs=msgu4[:, k, :],
                                     start=(jj == 0), stop=(jj == tpb - 1),
                                     skip_group_check=True)
                    if jj != tpb - 1:
                        continue
                    # ---------- phase 3 for block bb ----------
                    ss = pool.tile([128, H], dt.float32, tag="ss")
                    nc.vector.tensor_scalar(out=ss, in0=psb_cur[:, F:F2],
                                            scalar1=1e-30, scalar2=None,
                                            op0=AOP.max)
                    inv = pool.tile([128, H], dt.float32, tag="inv")
                    nc.vector.reciprocal(inv, ss)
                    mn = pool.tile([128, F], dt.float32, tag="mn")
                    nc.vector.tensor_tensor(
                        out=mn[:, :].rearrange("p (h d) -> p h d", h=H),
                        in0=psb_cur[:, 0:F].rearrange("p (h d) -> p h d", h=H),
                        in1=inv[:, :, None].broadcast_to((128, H, DH)),
                        op=AOP.mult)
                    psc = psC.tile([128, 128], dt.float32, tag="outC")
                    nc.tensor.matmul(psc, lhsT=w3_s,
                                     rhs=nftcm_s[:, bb * 128:(bb + 1) * 128],
                                     start=True, stop=False)
                    nc.tensor.matmul(psc, lhsT=mn, rhs=ident,
                                     is_transpose=True,
                                     start=False, stop=False)
                    nc.tensor.matmul(psc, lhsT=ident16,
                                     rhs=nftc_s[:, bb * 128:(bb + 1) * 128],
                                     start=False, stop=True)
                    oc2 = pool.tile([128, 128], dt.float16, tag="oc2")
                    nc.scalar.activation(oc2, psc,
                                         mybir.ActivationFunctionType.Relu)
                    nc.sync.dma_start(out=outT[:, bb * 128:(bb + 1) * 128],
                                      in_=oc2)

    nc.compile()
    return nc


def _binpack(deg, nbins, cap):
    """LPT bin-packing of nodes into nbins bins of <= cap nodes, minimizing
    the max per-bin degree sum. Returns (bin_of_node, slot_of_node, loads)."""
    n = len(deg)
    order = np.argsort(-deg, kind="stable")
    bin_of = np.empty(n, dtype=np.int64)
    slot_of = np.empty(n, dtype=np.int64)
    counts = np.zeros(nbins, dtype=np.int64)
    loads = np.zeros(nbins, dtype=np.int64)
    heap = [(0, i) for i in range(nbins)]
    heapq.heapify(heap)
    for nd in order:
        while True:
            load, b = heapq.heappop(heap)
            if counts[b] < cap:
                break
        bin_of[nd] = b
        slot_of[nd] = counts[b]
        counts[b] += 1
        loads[b] += deg[nd]
        if counts[b] < cap:
            heapq.heappush(heap, (loads[b], b))
    return bin_of, slot_of, loads


def prep_inputs(nft, eft, W_path, b_path, W_attn1, attn2, src, dst, npc):
    """Host-side sharding/relayout. Returns (in_maps, unperm, tpb, has_bias)."""
    n_nodes = nft.shape[0]
    nb = npc // 128
    nbins = NCORES * nb

    nft = np.ascontiguousarray(nft, dtype=np.float32)
    src = np.asarray(src, dtype=np.int64)
    dst = np.asarray(dst, dtype=np.int64)
    deg = np.bincount(dst, minlength=n_nodes)

    # --- node renumbering: balance per-block edge counts ---
    bin_of, slot_of, loads = _binpack(deg, nbins, 128)
    tpb = max(1, int(np.ceil(loads.max() / 128.0)))
    # assign bins to cores (LPT on loads, 20 bins per core)
    order = np.argsort(-loads, kind="stable")
    core_of_bin = np.empty(nbins, dtype=np.int64)
    block_of_bin = np.empty(nbins, dtype=np.int64)
    cheap = [(0, c) for c in range(NCORES)]
    heapq.heapify(cheap)
    ccount = np.zeros(NCORES, dtype=np.int64)
    for g in order:
        while True:
            load, c = heapq.heappop(cheap)
            if ccount[c] < nb:
                break
        core_of_bin[g] = c
        block_of_bin[g] = ccount[c]
        ccount[c] += 1
        load += loads[g]
        if ccount[c] < nb:
            heapq.heappush(cheap, (load, c))

    ntiles = nb * tpb
    epad = ntiles * 128
    npad = nbins * 128

    nft16 = nft.astype(np.float16)

    # attention combination weights
    a2 = np.asarray(attn2, dtype=np.float32).reshape(H, DH)
    A2blk = np.zeros((F, H), dtype=np.float32)
    for h in range(H):
        A2blk[h * DH:(h + 1) * DH, h] = a2[h]
    Wp = np.ascontiguousarray(W_path, dtype=np.float32)
    W1, W2, W3 = Wp[0:F], Wp[F:2 * F], Wp[2 * F:3 * F]
    w2cat = np.concatenate([W2, W2 @ A2blk], axis=1).astype(np.float16)
    w1qa = np.concatenate(
        [W1, np.asarray(W_attn1, np.float32) + W1 @ A2blk], axis=1
    ).astype(np.float16)
    w3_np = W3.astype(np.float16)

    has_bias = bool(np.any(np.asarray(b_path) != 0))
    b = np.asarray(b_path, dtype=np.float32).reshape(F)
    bqa = b @ A2blk  # folded into rdst
    # r[dst] logit term (y3@A2): tiny per-node table, gathered on host
    r_node = (nft @ (W3 @ A2blk)).astype(np.float32)  # [N, H]

    # --- edge routing ---
    core_of_e = core_of_bin[bin_of[dst]]
    sortkey = (core_of_e * nb + block_of_bin[bin_of[dst]]) * 128 + slot_of[dst]
    perm = np.argsort(sortkey, kind="stable")
    s_key = sortkey[perm]
    s_src = src[perm]
    s_dst = dst[perm]
    s_eft = np.asarray(eft)[perm]

    in_maps = []
    blkkey = s_key // 128  # global (core*nb + block) of each sorted edge
    for c in range(NCORES):
        eftT_c = np.zeros((F, epad), dtype=ml_dtypes.float8_e3m4)
        hsT_c = np.zeros((F, epad), dtype=np.float16)
        dstloc = np.zeros(epad, dtype=np.int64)
        rdst = np.full((epad, H), -10000.0, dtype=np.float32)

        for b_i in range(nb):
            gk = c * nb + b_i
            s = np.searchsorted(blkkey, gk)
            e = np.searchsorted(blkkey, gk + 1)
            cnt = e - s
            assert cnt <= tpb * 128, f"block overflow: {cnt} > {tpb * 128}"
            o = b_i * tpb * 128
            eftT_c[:, o:o + cnt] = s_eft[s:e].T.astype(ml_dtypes.float8_e3m4)
            hsT_c[:, o:o + cnt] = nft16[s_src[s:e]].T
            dstloc[o:o + cnt] = slot_of[s_dst[s:e]]
            rdst[o:o + cnt] = r_node[s_dst[s:e]] + bqa

        ee = np.arange(epad)
        Pcat_c = np.zeros((128, epad), dtype=mybir.dt.np(mybir.dt.float8e4))
        Pcat_c[ee % 128, (ee // 128) * 128 + dstloc] = 1.0
        rdst_cat = np.ascontiguousarray(
            rdst.reshape(ntiles, 128, H).transpose(1, 0, 2).reshape(128, ntiles * H)
        ).astype(np.float16)

        # per-core node features (new order)
        ids = np.arange(npc, dtype=np.int64)
        gbin = c * nb + ids // 128
        rows = np.zeros((npc, F), dtype=np.float32)
        mask = np.zeros(npc, dtype=bool)
        # nodes whose (bin) is assigned to this core at block ids//128
        for b_i in range(nb):
            # find global bin g with core_of_bin[g]==c and block_of_bin[g]==b_i
            g = np.where((core_of_bin == c) & (block_of_bin == b_i))[0][0]
            sel = bin_of == g
            nds = np.where(sel)[0]
            sl = slot_of[nds]
            rows[b_i * 128 + sl] = nft[nds]
            mask[b_i * 128 + sl] = deg[nds] > 0
        nftT_c = rows.T.astype(np.float16).copy()
        nftT_cm = (rows * mask[:, None]).T.astype(np.float16).copy()

        m = {
            "eftT": eftT_c,
            "hsT": hsT_c,
            "nftT_c": nftT_c,
            "nftT_cm": nftT_cm,
            "w2cat": w2cat,
            "w1qa": w1qa,
            "w3": w3_np,
            "Pcat": Pcat_c,
            "rdst": rdst_cat,
        }
        if has_bias:
            m["brow"] = b.astype(np.float16).reshape(1, F)
        in_maps.append(m)

    unperm = (core_of_bin, block_of_bin, bin_of, slot_of)
    return in_maps, unperm, tpb, has_bias


_NC_CACHE = {}


def _get_nc(key, *args, **kw):
    if key not in _NC_CACHE:
        _NC_CACHE[key] = build_nc(*args, **kw)
    return _NC_CACHE[key]


def run(nft, eft, W_path, b_path, W_attn1, attn2, src, dst, trace=False,
        tmpdir=None, prec="f16"):
    n_nodes = nft.shape[0]
    npc = ((n_nodes + NCORES - 1) // NCORES + 127) // 128 * 128
    nb = npc // 128

    in_maps, unperm, tpb, has_bias = prep_inputs(
        np.asarray(nft), np.asarray(eft), np.asarray(W_path),
        np.asarray(b_path), np.asarray(W_attn1), np.asarray(attn2),
        np.asarray(src), np.asarray(dst), npc)
    core_of_bin, block_of_bin, bin_of, slot_of = unperm
    npad = NCORES * nb * 128

    nc = _get_nc((npc, tpb, npad, has_bias), npc, tpb, npad, has_bias)
    kw = {}
    if trace:
        kw = dict(trace=True, tmpdir=tmpdir)
    res = bass_utils.run_bass_kernel_spmd(nc, in_maps,
                                          core_ids=list(range(NCORES)), **kw)

    out = np.empty((n_nodes, F), dtype=np.float32)
    cols = block_of_bin[bin_of] * 128 + slot_of  # column in owning core's outT
    cores = core_of_bin[bin_of]
    for c in range(NCORES):
        sel = cores == c
        out[sel] = res.results[c]["outT"][:, cols[sel]].T
    return out, res


def kernel(**inputs):
    out, _ = run(**inputs)
    return out
